# revision 1
# baseline (speedup 1.0000x reference)
"""CCNF RK4 sampling kernel for 8 Trainium2 NeuronCores.

Data-parallel: batch 2048 -> 256 per core, weights replicated.
On-core layout: features on partitions, batch on the free dim (N=256).
Matmuls run in float32r (fp32 data, fast PE mode).
"""

import os

import numpy as np

N_CORES = 8


def _build_program(theta0, context, W1, b1, W2, b2, W3, b3, n_steps):
    import concourse.bass as bass
    import concourse.mybir as mybir
    import concourse.tile as tile
    from concourse import bacc
    from concourse.bass_utils import run_bass_kernel_spmd

    f32 = mybir.dt.float32
    f32r = mybir.dt.float32r
    ALU = mybir.AluOpType
    SIGMOID = mybir.ActivationFunctionType.Sigmoid

    B, D = theta0.shape          # 2048, 32
    C = context.shape[1]         # 128
    IN, H2 = W1.shape            # 161, 1024
    H = W2.shape[0]              # 512
    assert H2 == 2 * H and W2.shape[1] == 2 * H and W3.shape == (H, D)
    assert IN == D + 1 + C
    assert B % N_CORES == 0
    Bs = B // N_CORES            # 256 per core
    steps = int(n_steps)
    dt = 1.0 / steps

    KC = H // 128                # 4 k-chunks for layer 2/3
    MJ = H // 128                # 4 output column-chunks per GLU half
    # layer-1 K split: rows [0:33) = theta(32)+t(1) (dynamic); rows [33:161) = ctx (static)
    K1A = D + 1                  # 33 (theta + t)
    K1B = IN - K1A               # 128 (ctx)

    # ---- host-side layout prep (shared across cores) ----
    W1 = np.ascontiguousarray(W1, np.float32)
    w1c1_h = np.ascontiguousarray(W1[:K1A])                    # [33, 1024]  theta+t rows
    w1c2_h = np.ascontiguousarray(W1[K1A:])                    # [128, 1024] ctx rows
    b3_is_zero = not np.any(np.asarray(b3, np.float32))
    # [512,1024] -> [128, 4*1024]: chunk kc at cols kc*1024
    w2_h = np.ascontiguousarray(
        np.asarray(W2, np.float32).reshape(KC, 128, 2 * H).transpose(1, 0, 2).reshape(128, KC * 2 * H)
    )
    # [512,32] -> [128, 4*32]
    w3_h = np.ascontiguousarray(
        np.asarray(W3, np.float32).reshape(KC, 128, D).transpose(1, 0, 2).reshape(128, KC * D)
    )
    b1 = np.asarray(b1, np.float32)
    b2 = np.asarray(b2, np.float32)
    bias_h = np.ascontiguousarray(np.concatenate([
        b1[:H].reshape(MJ, 128).T, b1[H:].reshape(MJ, 128).T,
        b2[:H].reshape(MJ, 128).T, b2[H:].reshape(MJ, 128).T,
    ], axis=1))                                                # [128, 16]
    onesb3_h = np.ascontiguousarray(np.concatenate([
        np.ones((1, Bs), np.float32),
        np.asarray(b3, np.float32).reshape(1, D),
    ], axis=1))                                                # [1, Bs+32]

    # ---- build the bass program (same program on all 8 cores) ----
    nc = bacc.Bacc("TRN2", target_bir_lowering=False)

    d_x1i = nc.dram_tensor("x1i", [K1A, Bs], f32r, kind="ExternalInput")  # theta rows + t row
    d_x2i = nc.dram_tensor("x2i", [K1B, Bs], f32r, kind="ExternalInput")  # ctx (static)
    d_th0 = nc.dram_tensor("th0", [D, Bs], f32r, kind="ExternalInput")
    d_w1c1 = nc.dram_tensor("w1c1", [K1A, 2 * H], f32r, kind="ExternalInput")
    d_w1c2 = nc.dram_tensor("w1c2", [K1B, 2 * H], f32r, kind="ExternalInput")
    d_w2 = nc.dram_tensor("w2", [128, KC * 2 * H], f32r, kind="ExternalInput")
    d_w3 = nc.dram_tensor("w3", [128, KC * D], f32r, kind="ExternalInput")
    d_bias = nc.dram_tensor("biases", [128, 4 * MJ], f32, kind="ExternalInput")
    d_ob3 = nc.dram_tensor("onesb3", [1, Bs + D], f32r, kind="ExternalInput")
    d_out = nc.dram_tensor("out", [D, Bs], f32, kind="ExternalOutput")

    # RK4 coefficients: arg scale (for next eval's input), acc scale
    c_arg = [0.5 * dt, 0.5 * dt, dt]
    a_acc = [dt / 6.0, dt / 3.0, dt / 3.0, dt / 6.0]

    with tile.TileContext(nc) as tc:
        PS3_SHARE = int(os.environ.get("KERNEL_PS3SHARE", "0"))
        PSMM_BUFS = 8 if PS3_SHARE else 7
        with (
            tc.tile_pool(name="const", bufs=1) as cpool,
            tc.tile_pool(name="psmm", bufs=PSMM_BUFS, space="PSUM") as ps_pool,
            tc.tile_pool(name="ps3", bufs=1, space="PSUM") as ps3_pool,
            tc.tile_pool(name="sig", bufs=int(os.environ.get("KERNEL_SIGB", "10"))) as sig_pool,
            tc.tile_pool(name="hact", bufs=int(os.environ.get("KERNEL_HB", "20"))) as h_pool,
            tc.tile_pool(name="accp", bufs=int(os.environ.get("KERNEL_AB", "6"))) as acc_pool,
        ):
            tw1c1 = cpool.tile([K1A, 2 * H], f32r)
            tw1c2 = cpool.tile([K1B, 2 * H], f32r)
            tw2 = cpool.tile([128, KC * 2 * H], f32r)
            tw3 = cpool.tile([128, KC * D], f32r)
            tbias = cpool.tile([128, 4 * MJ], f32)
            tb1a = tbias[:, 0 * MJ : 1 * MJ]
            tb1b = tbias[:, 1 * MJ : 2 * MJ]
            tb2a = tbias[:, 2 * MJ : 3 * MJ]
            tb2b = tbias[:, 3 * MJ : 4 * MJ]
            tob3 = cpool.tile([1, Bs + D], f32r)
            tones = tob3[:, 0:Bs]
            tb3 = tob3[:, Bs : Bs + D]
            tx1 = cpool.tile([K1A, Bs], f32r)   # rows: [theta(32) | t(1)]  (dynamic)
            tx2 = cpool.tile([K1B, Bs], f32r)   # ctx (static)
            tth0 = cpool.tile([D, Bs], f32r)    # initial theta

            # layer-1-critical tensors first so eval 0 can start while
            # w2/w3 still stream
            nc.sync.dma_start(tx2[:], d_x2i[:])
            nc.sync.dma_start(tw1c2[:], d_w1c2[:])
            nc.sync.dma_start(tx1[:], d_x1i[:])
            nc.sync.dma_start(tw1c1[:], d_w1c1[:])
            nc.sync.dma_start(tbias[:], d_bias[:])
            nc.sync.dma_start(tth0[:], d_th0[:])
            nc.sync.dma_start(tw2[:], d_w2[:])
            nc.sync.dma_start(tw3[:], d_w3[:])
            nc.sync.dma_start(tob3[:], d_ob3[:])

            def mm(out_ap, lhsT_ap, rhs_ap, start, stop):
                nc.tensor.matmul(out_ap, lhsT_ap, rhs_ap, start=start, stop=stop)

            th_cur = tth0       # theta at start of current step
            t_written = 0.0     # t-row was preloaded with 0

            def issue_l1ctx():
                # static context contribution for the NEXT eval's layer 1 --
                # issued early so PE has work during the RK4 latency chain.
                # One accumulation group per PSUM bank: only the first MM may
                # carry start=True (a second start would zero the whole bank).
                tiles = []
                for j in range(MJ):
                    ps = ps_pool.tile([128, 2 * Bs], f32, tag="psmm")
                    for half, mj in ((1, j + MJ), (0, j)):
                        dst = ps[:, half * Bs : (half + 1) * Bs]
                        msl = slice(mj * 128, (mj + 1) * 128)
                        mm(dst, tw1c2[:, msl], tx2[:],
                           start=(half == 1), stop=False)
                    tiles.append(ps)
                return tiles

            SCHED = int(os.environ.get("KERNEL_SCHED", "3"))
            SPLITP = int(os.environ.get("KERNEL_SPLITPOOLS", "1"))
            SIG1T, SIG2T = ("sig1", "sig2") if SPLITP else ("sig", "sig")
            H1T, H2T = ("h1t", "h2t") if SPLITP else ("hact", "hact")
            if SCHED >= 2 or SCHED == 3:
                ps1 = issue_l1ctx()

            TOFF = (0.0, 0.5, 0.5, 1.0)
            for s in range(steps):
                for e in range(4):
                    TMEMSET_MODE = int(os.environ.get("KERNEL_TMEMSET", "0")) if SCHED == 3 else 0
                    TMEMSET_TAIL = TMEMSET_MODE == 1
                    TMEMSET_POST = TMEMSET_MODE == 2
                    TMEMSET_POOL_TAIL = TMEMSET_MODE == 3
                    t_val = (s + TOFF[e]) * dt
                    if TMEMSET_MODE == 0 and t_val != t_written:
                        nc.gpsimd.memset(tx1[D : D + 1, :].bitcast(f32), float(t_val))
                        t_written = t_val

                    last_eval = (s == steps - 1) and (e == 3)

                    if SCHED == 3:
                        # v1.5 structure, ctx MMs pre-issued (ps1 tiles);
                        # group stop goes on the LAST MM of the bank (a-half)
                        h1 = []
                        for j in range(MJ):
                            ps = ps1[j]
                            for half, mj in ((1, j + MJ), (0, j)):
                                dst = ps[:, half * Bs : (half + 1) * Bs]
                                msl = slice(mj * 128, (mj + 1) * 128)
                                mm(dst, tw1c1[:, msl], tx1[:], start=False,
                                   stop=(half == 0))
                            sg = sig_pool.tile([128, Bs], f32, tag=SIG1T)
                            nc.scalar.activation(
                                sg[:], ps[:, Bs : 2 * Bs], SIGMOID,
                                bias=tb1b[:, j : j + 1]
                            )
                            ht = h_pool.tile([128, Bs], f32r, tag=H1T)
                            nc.vector.scalar_tensor_tensor(
                                ht[:], ps[:, 0:Bs], tb1a[:, j : j + 1], sg[:],
                                ALU.add, ALU.mult,
                            )
                            h1.append(ht)
                        h2 = []
                        for j in range(MJ):
                            ps = ps_pool.tile([128, 2 * Bs], f32, tag="psmm")
                            # b-half group first so the sigmoid overlaps the
                            # a-half matmuls (keeps ACT off the eval tail)
                            dstb = ps[:, Bs : 2 * Bs]
                            for kc in range(KC):
                                csl = slice(kc * 2 * H + (j + MJ) * 128,
                                            kc * 2 * H + (j + MJ + 1) * 128)
                                mm(dstb, tw2[:, csl], h1[kc][:],
                                   start=(kc == 0), stop=(kc == KC - 1))
                            sg = sig_pool.tile([128, Bs], f32, tag=SIG2T)
                            nc.scalar.activation(
                                sg[:], dstb, SIGMOID, bias=tb2b[:, j : j + 1]
                            )
                            dsta = ps[:, 0:Bs]
                            for kc in range(KC):
                                csl = slice(kc * 2 * H + j * 128,
                                            kc * 2 * H + (j + 1) * 128)
                                mm(dsta, tw2[:, csl], h1[kc][:],
                                   start=(kc == 0), stop=(kc == KC - 1))
                            ht = h_pool.tile([128, Bs], f32r, tag=H2T)
                            nc.vector.scalar_tensor_tensor(
                                ht[:], dsta, tb2a[:, j : j + 1], sg[:],
                                ALU.add, ALU.mult,
                            )
                            h2.append(ht)
                        if not last_eval:
                            ps1_next = issue_l1ctx()
                    elif SCHED == 0:
                        # v1.5: per-j, both halves, GLU immediately
                        h1 = []
                        for j in range(MJ):
                            ps = ps_pool.tile([128, 2 * Bs], f32, tag="psmm")
                            for half, mj in ((0, j), (1, j + MJ)):
                                dst = ps[:, half * Bs : (half + 1) * Bs]
                                msl = slice(mj * 128, (mj + 1) * 128)
                                mm(dst, tw1c2[:, msl], tx2[:], start=True, stop=False)
                                mm(dst, tw1c1[:, msl], tx1[:], start=False, stop=True)
                            sg = sig_pool.tile([128, Bs], f32, tag="sig")
                            nc.scalar.activation(
                                sg[:], ps[:, Bs : 2 * Bs], SIGMOID,
                                bias=tb1b[:, j : j + 1]
                            )
                            ht = h_pool.tile([128, Bs], f32r, tag="hact")
                            nc.vector.scalar_tensor_tensor(
                                ht[:], ps[:, 0:Bs], tb1a[:, j : j + 1], sg[:],
                                ALU.add, ALU.mult,
                            )
                            h1.append(ht)
                        h2 = []
                        for j in range(MJ):
                            ps = ps_pool.tile([128, 2 * Bs], f32, tag="psmm")
                            for half, mj in ((0, j), (1, j + MJ)):
                                dst = ps[:, half * Bs : (half + 1) * Bs]
                                for kc in range(KC):
                                    csl = slice(kc * 2 * H + mj * 128,
                                                kc * 2 * H + (mj + 1) * 128)
                                    mm(dst, tw2[:, csl], h1[kc][:],
                                       start=(kc == 0), stop=(kc == KC - 1))
                            sg = sig_pool.tile([128, Bs], f32, tag="sig")
                            nc.scalar.activation(
                                sg[:], ps[:, Bs : 2 * Bs], SIGMOID,
                                bias=tb2b[:, j : j + 1]
                            )
                            ht = h_pool.tile([128, Bs], f32r, tag="hact")
                            nc.vector.scalar_tensor_tensor(
                                ht[:], ps[:, 0:Bs], tb2a[:, j : j + 1], sg[:],
                                ALU.add, ALU.mult,
                            )
                            h2.append(ht)
                    else:
                        if SCHED < 2:
                            ps1 = issue_l1ctx()
                        sg1 = []
                        for j in range(MJ):
                            dst = ps1[j][:, Bs : 2 * Bs]
                            msl = slice((j + MJ) * 128, (j + MJ + 1) * 128)
                            mm(dst, tw1c1[:, msl], tx1[:], start=False, stop=True)
                            sg = sig_pool.tile([128, Bs], f32, tag="sig")
                            nc.scalar.activation(
                                sg[:], dst, SIGMOID, bias=tb1b[:, j : j + 1]
                            )
                            sg1.append(sg)
                        h1 = []
                        for j in range(MJ):
                            dst = ps1[j][:, 0:Bs]
                            msl = slice(j * 128, (j + 1) * 128)
                            mm(dst, tw1c1[:, msl], tx1[:], start=False, stop=True)
                            ht = h_pool.tile([128, Bs], f32r, tag="hact")
                            nc.vector.scalar_tensor_tensor(
                                ht[:], dst, tb1a[:, j : j + 1], sg1[j][:],
                                ALU.add, ALU.mult,
                            )
                            h1.append(ht)
                        ps2 = []
                        for j in range(MJ):
                            ps2j = ps_pool.tile([128, 2 * Bs], f32, tag="psmm")
                            ps2.append(ps2j)
                        for kc in range(KC - 1):
                            for j in range(MJ):
                                csl = slice(kc * 2 * H + (j + MJ) * 128,
                                            kc * 2 * H + (j + MJ + 1) * 128)
                                mm(ps2[j][:, Bs : 2 * Bs], tw2[:, csl], h1[kc][:],
                                   start=(kc == 0), stop=False)
                            for j in range(MJ):
                                csl = slice(kc * 2 * H + j * 128,
                                            kc * 2 * H + (j + 1) * 128)
                                mm(ps2[j][:, 0:Bs], tw2[:, csl], h1[kc][:],
                                   start=(kc == 0), stop=False)
                        kc = KC - 1
                        sg2 = []
                        for j in range(MJ):
                            csl = slice(kc * 2 * H + (j + MJ) * 128,
                                        kc * 2 * H + (j + MJ + 1) * 128)
                            dst = ps2[j][:, Bs : 2 * Bs]
                            mm(dst, tw2[:, csl], h1[kc][:], start=False, stop=True)
                            sg = sig_pool.tile([128, Bs], f32, tag="sig")
                            nc.scalar.activation(
                                sg[:], dst, SIGMOID, bias=tb2b[:, j : j + 1]
                            )
                            sg2.append(sg)
                        h2 = []
                        for j in range(MJ):
                            csl = slice(kc * 2 * H + j * 128,
                                        kc * 2 * H + (j + 1) * 128)
                            dst = ps2[j][:, 0:Bs]
                            mm(dst, tw2[:, csl], h1[kc][:], start=False, stop=True)
                            ht = h_pool.tile([128, Bs], f32r, tag="hact")
                            nc.vector.scalar_tensor_tensor(
                                ht[:], dst, tb2a[:, j : j + 1], sg2[j][:],
                                ALU.add, ALU.mult,
                            )
                            h2.append(ht)
                        if SCHED >= 2 and not last_eval:
                            ps1_next = issue_l1ctx()

                    # ---- layer 3: k = h2 @ W3 (+ b3) in PSUM ----
                    if PS3_SHARE:
                        ps3full = ps_pool.tile([128, 2 * Bs], f32, tag="psmm")
                        ps3 = ps3full[0:D, 0:Bs]
                    else:
                        ps3 = ps3_pool.tile([D, Bs], f32, tag="ps3")
                    for kc in range(KC):
                        mm(ps3[:], tw3[:, kc * D : (kc + 1) * D], h2[kc][:],
                           start=(kc == 0), stop=(kc == KC - 1 and b3_is_zero))
                    if not b3_is_zero:
                        mm(ps3[:], tb3[:], tones[:], start=False, stop=True)

                    # ---- RK4 bookkeeping ----
                    if TMEMSET_POOL_TAIL and not last_eval:
                        nxt_s, nxt_e = (s, e + 1) if e < 3 else (s + 1, 0)
                        nxt_t = (nxt_s + TOFF[nxt_e]) * dt
                        if nxt_t != t_written:
                            nc.gpsimd.memset(
                                tx1[D : D + 1, :].bitcast(f32), float(nxt_t)
                            )
                            t_written = nxt_t
                    if TMEMSET_TAIL and not last_eval:
                        # write the NEXT eval's t-row on DVE (same engine as
                        # the arg STT -> no extra cross-engine hop on the
                        # arg -> layer-1 chain)
                        nxt_s, nxt_e = (s, e + 1) if e < 3 else (s + 1, 0)
                        nxt_t = (nxt_s + TOFF[nxt_e]) * dt
                        if nxt_t != t_written:
                            nc.vector.memset(
                                tx1[D : D + 1, :].bitcast(f32), float(nxt_t)
                            )
                            t_written = nxt_t
                    if e < 3:
                        nc.vector.scalar_tensor_tensor(
                            tx1[0:D, :], ps3[:], float(c_arg[e]), th_cur[:],
                            ALU.mult, ALU.add,
                        )
                    if TMEMSET_POST and not last_eval:
                        nxt_s, nxt_e = (s, e + 1) if e < 3 else (s + 1, 0)
                        nxt_t = (nxt_s + TOFF[nxt_e]) * dt
                        if nxt_t != t_written:
                            nc.vector.memset(
                                tx1[D : D + 1, :].bitcast(f32), float(nxt_t)
                            )
                            t_written = nxt_t
                    base = th_cur if e == 0 else acc_prev
                    if e == 3 and s != steps - 1:
                        # theta_{s+1} goes straight into the matmul input tile
                        # (keeps the Pool copy off the critical chain)...
                        nc.vector.scalar_tensor_tensor(
                            tx1[0:D, :], ps3[:], float(a_acc[e]), base[:],
                            ALU.mult, ALU.add,
                        )
                    acc_new = acc_pool.tile([D, Bs], f32, tag="accp")
                    # ...and also into its own tile (used as th_cur next step)
                    nc.vector.scalar_tensor_tensor(
                        acc_new[:], ps3[:], float(a_acc[e]), base[:],
                        ALU.mult, ALU.add,
                    )
                    acc_prev = acc_new
                    if SCHED >= 2 and not last_eval:
                        ps1 = ps1_next

                th_cur = acc_prev  # theta_{s+1}

            nc.sync.dma_start(d_out[:], th_cur[:])

    # ---- per-core input maps ----
    in_maps = []
    for c in range(N_CORES):
        sl = slice(c * Bs, (c + 1) * Bs)
        th_T = np.ascontiguousarray(np.asarray(theta0[sl], np.float32).T)
        ctx_T = np.ascontiguousarray(np.asarray(context[sl], np.float32).T)
        x1i = np.concatenate([th_T, np.zeros((1, Bs), np.float32)], axis=0)
        in_maps.append(
            {
                "x1i": np.ascontiguousarray(x1i),
                "x2i": ctx_T,
                "th0": th_T,
                "w1c1": w1c1_h,
                "w1c2": w1c2_h,
                "w2": w2_h,
                "w3": w3_h,
                "biases": bias_h,
                "onesb3": onesb3_h,
            }
        )

    return nc, in_maps


def _build_and_run(theta0, context, W1, b1, W2, b2, W3, b3, n_steps):
    from concourse.bass_utils import run_bass_kernel_spmd

    nc, in_maps = _build_program(theta0, context, W1, b1, W2, b2, W3, b3, n_steps)
    nc.finalize()  # Bacc: split multi-sem waits + allocate registers
    res = run_bass_kernel_spmd(
        nc,
        in_maps,
        core_ids=list(range(N_CORES)),
        trace=bool(int(os.environ.get("KERNEL_TRACE", "0"))),
    )
    _build_and_run.last_results = res

    out = np.concatenate([r["out"].T for r in res.results], axis=0)
    return np.ascontiguousarray(out.astype(np.float32))


def kernel(theta0, context, W1, b1, W2, b2, W3, b3, n_steps):
    return _build_and_run(
        np.asarray(theta0), np.asarray(context), W1, b1, W2, b2, W3, b3, n_steps
    )



# revision 24
# speedup vs baseline: 1.3274x; 1.3274x over previous
"""CCNF RK4 sampling kernel for 8 Trainium2 NeuronCores.

Data-parallel: batch 2048 -> 256 per core, weights replicated.
On-core layout: features on partitions, batch on the free dim (N=256).
Matmuls run in float32r (fp32 data, fast PE mode).
"""

import os

import numpy as np

N_CORES = 8


def _build_program_fast(theta0, context, W1, b1, W2, b2, W3, b3, n_steps):
    """Two-stream fast path (requires zero biases).

    Per core: batch 256 split into two independent 128-sample streams so
    each stream's sigmoid/GLU chain overlaps the other stream's matmuls.
    Layer 1 and 3 run in bf16, layer 2 in fp8e4m3 with DoubleRow (2x PE).
    """
    import ml_dtypes

    import concourse.mybir as mybir
    import concourse.tile as tile
    from concourse import bacc

    f32 = mybir.dt.float32
    bf16 = mybir.dt.bfloat16
    f8 = mybir.dt.float8e4
    ALU = mybir.AluOpType
    SIGMOID = mybir.ActivationFunctionType.Sigmoid
    DR = mybir.MatmulPerfMode.DoubleRow

    np_bf16 = ml_dtypes.bfloat16
    np_f8 = ml_dtypes.float8_e4m3

    B, D = theta0.shape          # 2048, 32
    C = context.shape[1]         # 128
    IN, H2 = W1.shape            # 161, 1024
    H = W2.shape[0]              # 512
    assert H2 == 2 * H and W2.shape[1] == 2 * H and W3.shape == (H, D)
    assert IN == D + 1 + C and D == 32 and C == 128 and H == 512
    assert B % (N_CORES * 2) == 0
    Bs = B // N_CORES            # 256 per core
    Ns = Bs // 2                 # 128 per stream
    steps = int(n_steps)
    dtv = 1.0 / steps

    S_H1 = 16.0                  # h1 tiles carry 16*h1 in fp8
    S_W2 = 32.0                  # W2 stored as 32*W2 in fp8

    # ---- host-side packing (shared across cores) ----
    W1 = np.asarray(W1, np.float32)
    w1a_h = np.ascontiguousarray(W1[: D + 1]).astype(np_bf16)      # [33,1024]
    w1c_h = np.ascontiguousarray(W1[D + 1 :]).astype(np_bf16)      # [128,1024]
    # W2 -> DoubleRow pairs: w2p[p, i, g*2H + m] = 32*W2[g*256 + i*128 + p, m]
    w2 = np.asarray(W2, np.float32) * S_W2
    w2p_h = np.ascontiguousarray(
        w2.reshape(2, 2, 128, 2 * H).transpose(2, 1, 0, 3).reshape(128, 4 * 2 * H)
    ).astype(np_f8)                                                # [128, 8192]
    # W3 -> [128, KC*D]: cols kc*D+d = W3[kc*128+p, d]
    w3_h = np.ascontiguousarray(
        np.asarray(W3, np.float32).reshape(4, 128, D).transpose(1, 0, 2).reshape(128, 4 * D)
    ).astype(np_bf16)

    c_arg = [0.5 * dtv, 0.5 * dtv, dtv]
    a_acc = [dtv / 6.0, dtv / 3.0, dtv / 3.0, dtv / 6.0]
    TOFF = (0.0, 0.5, 0.5, 1.0)

    nc = bacc.Bacc("TRN2", target_bir_lowering=False)

    d_x1 = nc.dram_tensor("x1", [D + 1, Bs], bf16, kind="ExternalInput")
    d_ctx = nc.dram_tensor("ctx", [C, Bs], bf16, kind="ExternalInput")
    d_th0 = nc.dram_tensor("th0", [D, Bs], f32, kind="ExternalInput")
    d_w1a = nc.dram_tensor("w1a", [D + 1, 2 * H], bf16, kind="ExternalInput")
    d_w1c = nc.dram_tensor("w1c", [C, 2 * H], bf16, kind="ExternalInput")
    d_w2p = nc.dram_tensor("w2p", [128, 4 * 2 * H], f8, kind="ExternalInput")
    d_w3 = nc.dram_tensor("w3", [128, 4 * D], bf16, kind="ExternalInput")
    d_out = nc.dram_tensor("out", [D, Bs], f32, kind="ExternalOutput")

    with tile.TileContext(nc) as tc:
        
        with (
            tc.tile_pool(name="const", bufs=1) as cpool,
            tc.tile_pool(name="psl1", bufs=3, space="PSUM") as ps1_pool,
            tc.tile_pool(name="psl2", bufs=3, space="PSUM") as ps2_pool,
            tc.tile_pool(name="ps3a", bufs=1, space="PSUM") as ps3a_pool,
            tc.tile_pool(name="ps3b", bufs=1, space="PSUM") as ps3b_pool,
            tc.tile_pool(name="sig", bufs=int(os.environ.get("KERNEL_SIGB", "8"))) as sig_pool,
            tc.tile_pool(name="hact", bufs=int(os.environ.get("KERNEL_HB", "12"))) as h_pool,
            tc.tile_pool(name="accp", bufs=int(os.environ.get("KERNEL_AB", "8"))) as acc_pool,
        ):
            tw1a = cpool.tile([D + 1, 2 * H], bf16)
            tw1c = cpool.tile([C, 2 * H], bf16)
            tw2p = cpool.tile([128, 2, 2 * 2 * H], f8)  # [p, i, g*2H+m]
            tw3 = cpool.tile([128, 4 * D], bf16)
            tctx = cpool.tile([C, Bs], bf16)            # both streams
            txp = [[cpool.tile([D + 1, Ns], bf16, name=f"txp{u}_{p}") for p in range(2)]
                   for u in range(2)]
            tth0 = cpool.tile([D, Bs], f32)
            twz = cpool.tile([1, 128], bf16)

            # warmup: burn the PE p-state ramp while input DMAs stream
            nc.gpsimd.memset(twz[:], 0.0)
            NWARM = int(os.environ.get("KERNEL_WARM", "40"))
            if NWARM:
                psw = ps1_pool.tile([128, 2 * Bs], f32, tag="psmm", name="psw")
                for i in range(NWARM):
                    nc.tensor.matmul(psw[:, 0:128], twz[:], twz[:],
                                     start=(i == 0), stop=(i == NWARM - 1))

            # l1-critical tensors first (dispatch order == arrival order)
            nc.sync.dma_start(txp[0][0][:], d_x1[:, 0:Ns])
            nc.sync.dma_start(txp[1][0][:], d_x1[:, Ns:Bs])
            nc.sync.dma_start(tw1a[:], d_w1a[:])
            nc.sync.dma_start(tctx[:], d_ctx[:])
            nc.sync.dma_start(tw1c[:], d_w1c[:])
            nc.sync.dma_start(tw2p[:], d_w2p[:])
            nc.sync.dma_start(tth0[:], d_th0[:])
            nc.sync.dma_start(tw3[:], d_w3[:])

            def mm(out_ap, lhsT_ap, rhs_ap, start, stop, pm=None):
                nc.tensor.matmul(out_ap, lhsT_ap, rhs_ap, start=start,
                                 stop=stop, perf_mode=pm)

            # W1 column base per bank: A01, A23, B01, B23
            L1_BANKS = ((0, "A01"), (256, "A23"), (512, "B01"), (768, "B23"))

            th_cur = [tth0[:, 0:Ns], tth0[:, Ns:Bs]]
            acc_prev = [None, None]

            def alloc_l1_banks():
                bb, ba = [], []
                for u in range(2):
                    tb = ps1_pool.tile([128, 2 * Bs], f32, tag="psmm", name=f"l1B{u}")
                    ta = ps1_pool.tile([128, 2 * Bs], f32, tag="psmm", name=f"l1A{u}")
                    bb.append(tb)
                    ba.append(ta)
                return bb, ba

            def emit_ctx(bb, ba):
                # ctx half of layer 1 for the NEXT eval: tx-independent,
                # fills PE idle during the current eval's tail
                for u in range(2):
                    ctxu = tctx[:, u * Ns : (u + 1) * Ns]
                    for jb in range(4):
                        mm(bb[u][:, jb * 128 : (jb + 1) * 128],
                           tw1c[:, H + jb * 128 : H + (jb + 1) * 128],
                           ctxu, start=(jb == 0), stop=False)
                    for ja in range(4):
                        mm(ba[u][:, ja * 128 : (ja + 1) * 128],
                           tw1c[:, ja * 128 : (ja + 1) * 128],
                           ctxu, start=(ja == 0), stop=False)

            nbanks = alloc_l1_banks()
            emit_ctx(*nbanks)

            for s in range(steps):
                for e in range(4):
                    last_eval = (s == steps - 1) and (e == 3)
                    par = (4 * s + e) % 2
                    bankB, bankA = nbanks

                    # ---- layer 1 theta MMs + GLU per stream
                    hp = [None, None]
                    for u in range(2):
                        for jb in range(4):
                            mm(bankB[u][:, jb * 128 : (jb + 1) * 128],
                               tw1a[:, H + jb * 128 : H + (jb + 1) * 128],
                               txp[u][par][:], start=False, stop=(jb == 3))
                        sg1 = sig_pool.tile([128, 2 * Bs], f32, tag=f"sg{u}", name=f"sg1_{u}")
                        nc.scalar.activation(sg1[:, 0:Bs], bankB[u][:, 0:Bs], SIGMOID)
                        nc.scalar.activation(sg1[:, Bs : 2 * Bs],
                                             bankB[u][:, Bs : 2 * Bs], SIGMOID)
                        for ja in range(4):
                            mm(bankA[u][:, ja * 128 : (ja + 1) * 128],
                               tw1a[:, ja * 128 : (ja + 1) * 128],
                               txp[u][par][:], start=False, stop=(ja == 3))
                        ht = h_pool.tile([128, 4, 128], f8, tag=f"hp{u}", name=f"hp{u}")
                        nc.vector.scalar_tensor_tensor(
                            ht[:, 0:2, :], bankA[u][:, 0:Bs], S_H1,
                            sg1[:, 0:Bs], ALU.mult, ALU.mult,
                        )
                        nc.vector.scalar_tensor_tensor(
                            ht[:, 2:4, :], bankA[u][:, Bs : 2 * Bs], S_H1,
                            sg1[:, Bs : 2 * Bs], ALU.mult, ALU.mult,
                        )
                        hp[u] = ht

                    # ---- layer 2 per stream
                    hq = [None, None]
                    bases = [th_cur[u] if e == 0 else acc_prev[u][:] for u in range(2)]
                    for u in range(2):
                        b2B = ps2_pool.tile([128, 2 * Bs], f32, tag="psmm", name=f"l2B{u}")
                        b2A = ps2_pool.tile([128, 2 * Bs], f32, tag="psmm", name=f"l2A{u}")
                        for g in range(2):
                            for jb in range(4):
                                mj = 4 + jb
                                csl = slice(g * 2 * H + mj * 128, g * 2 * H + (mj + 1) * 128)
                                mm(b2B[:, jb * 128 : (jb + 1) * 128],
                                   tw2p[:, :, csl], hp[u][:, 2 * g : 2 * g + 2, :],
                                   start=(g == 0 and jb == 0), stop=(g == 1 and jb == 3),
                                   pm=DR)
                        sg2 = sig_pool.tile([128, 2 * Bs], f32, tag=f"s2{u}", name=f"sg2_{u}")
                        nc.scalar.activation(
                            sg2[:, 0:Bs], b2B[:, 0:Bs], SIGMOID,
                            scale=1.0 / (S_H1 * S_W2),
                        )
                        nc.scalar.activation(
                            sg2[:, Bs : 2 * Bs], b2B[:, Bs : 2 * Bs], SIGMOID,
                            scale=1.0 / (S_H1 * S_W2),
                        )
                        for g in range(2):
                            for ja in range(4):
                                csl = slice(g * 2 * H + ja * 128, g * 2 * H + (ja + 1) * 128)
                                mm(b2A[:, ja * 128 : (ja + 1) * 128],
                                   tw2p[:, :, csl], hp[u][:, 2 * g : 2 * g + 2, :],
                                   start=(g == 0 and ja == 0), stop=(g == 1 and ja == 3),
                                   pm=DR)
                        ht = h_pool.tile([128, 4, 128], bf16, tag=f"hq{u}", name=f"hq{u}")
                        nc.vector.scalar_tensor_tensor(
                            ht[:, 0:2, :], b2A[:, 0:Bs],
                            1.0 / (S_H1 * S_W2), sg2[:, 0:Bs],
                            ALU.mult, ALU.mult,
                        )
                        nc.vector.scalar_tensor_tensor(
                            ht[:, 2:4, :], b2A[:, Bs : 2 * Bs],
                            1.0 / (S_H1 * S_W2), sg2[:, Bs : 2 * Bs],
                            ALU.mult, ALU.mult,
                        )
                        hq[u] = ht

                    # ---- ctx MMs of the NEXT eval (PE filler during tail)
                    if not last_eval:
                        nbanks = alloc_l1_banks()
                        emit_ctx(*nbanks)

                    # ---- layer 3 + tx per stream
                    ps3s = [None, None]
                    for u in range(2):
                        ps3s[u] = (ps3a_pool if u == 0 else ps3b_pool).tile(
                            [D, Ns], f32, tag=f"ps3{u}", name=f"ps3{u}")
                        for kc in range(4):
                            mm(ps3s[u][:],
                               tw3[:, kc * D : (kc + 1) * D],
                               hq[u][:, kc : kc + 1, :],
                               start=(kc == 0), stop=(kc == 3))
                        nxt = txp[u][1 - par]
                        if e < 3:
                            nc.vector.scalar_tensor_tensor(
                                nxt[0:D, :], ps3s[u][:], float(c_arg[e]), th_cur[u],
                                ALU.mult, ALU.add,
                            )
                        elif not last_eval:
                            nc.vector.scalar_tensor_tensor(
                                nxt[0:D, :], ps3s[u][:], float(a_acc[e]), bases[u],
                                ALU.mult, ALU.add,
                            )
                        if not last_eval:
                            nxt_s, nxt_e = (s, e + 1) if e < 3 else (s + 1, 0)
                            nxt_t = (nxt_s + TOFF[nxt_e]) * dtv
                            nc.gpsimd.memset(nxt[D : D + 1, :], float(nxt_t))

                    for u in range(2):
                        acc_new = acc_pool.tile([D, Ns], f32, tag=f"acc{u}", name=f"acc{u}")
                        nc.vector.scalar_tensor_tensor(
                            acc_new[:], ps3s[u][:], float(a_acc[e]), bases[u],
                            ALU.mult, ALU.add,
                        )
                        acc_prev[u] = acc_new

                for u in range(2):
                    th_cur[u] = acc_prev[u][:]

            for u in range(2):
                nc.sync.dma_start(d_out[:, u * Ns : (u + 1) * Ns], acc_prev[u][:])

    # ---- per-core input maps ----
    in_maps = []
    for c in range(N_CORES):
        sl = slice(c * Bs, (c + 1) * Bs)
        th_T = np.ascontiguousarray(np.asarray(theta0[sl], np.float32).T)
        ctx_T = np.ascontiguousarray(np.asarray(context[sl], np.float32).T)
        x1 = np.concatenate([th_T, np.zeros((1, Bs), np.float32)], axis=0)
        in_maps.append(
            {
                "x1": np.ascontiguousarray(x1.astype(np_bf16)),
                "ctx": np.ascontiguousarray(ctx_T.astype(np_bf16)),
                "th0": th_T,
                "w1a": w1a_h,
                "w1c": w1c_h,
                "w2p": w2p_h,
                "w3": w3_h,
            }
        )
    return nc, in_maps


def _build_program(theta0, context, W1, b1, W2, b2, W3, b3, n_steps):
    if (
        int(os.environ.get("KERNEL_FAST", "1"))
        and not np.any(np.asarray(b1))
        and not np.any(np.asarray(b2))
        and not np.any(np.asarray(b3))
        and theta0.shape == (2048, 32)
        and int(n_steps) == 16
    ):
        return _build_program_fast(
            theta0, context, W1, b1, W2, b2, W3, b3, n_steps
        )
    return _build_program_v1(theta0, context, W1, b1, W2, b2, W3, b3, n_steps)


def _build_program_v1(theta0, context, W1, b1, W2, b2, W3, b3, n_steps):
    import concourse.bass as bass
    import concourse.mybir as mybir
    import concourse.tile as tile
    from concourse import bacc
    from concourse.bass_utils import run_bass_kernel_spmd

    f32 = mybir.dt.float32
    f32r = mybir.dt.float32r
    ALU = mybir.AluOpType
    SIGMOID = mybir.ActivationFunctionType.Sigmoid

    B, D = theta0.shape          # 2048, 32
    C = context.shape[1]         # 128
    IN, H2 = W1.shape            # 161, 1024
    H = W2.shape[0]              # 512
    assert H2 == 2 * H and W2.shape[1] == 2 * H and W3.shape == (H, D)
    assert IN == D + 1 + C
    assert B % N_CORES == 0
    Bs = B // N_CORES            # 256 per core
    steps = int(n_steps)
    dt = 1.0 / steps

    KC = H // 128                # 4 k-chunks for layer 2/3
    MJ = H // 128                # 4 output column-chunks per GLU half
    # layer-1 K split: rows [0:33) = theta(32)+t(1) (dynamic); rows [33:161) = ctx (static)
    K1A = D + 1                  # 33 (theta + t)
    K1B = IN - K1A               # 128 (ctx)

    # ---- host-side layout prep (shared across cores) ----
    W1 = np.ascontiguousarray(W1, np.float32)
    w1c1_h = np.ascontiguousarray(W1[:K1A])                    # [33, 1024]  theta+t rows
    w1c2_h = np.ascontiguousarray(W1[K1A:])                    # [128, 1024] ctx rows
    b3_is_zero = not np.any(np.asarray(b3, np.float32))
    # [512,1024] -> [128, 4*1024]: chunk kc at cols kc*1024
    w2_h = np.ascontiguousarray(
        np.asarray(W2, np.float32).reshape(KC, 128, 2 * H).transpose(1, 0, 2).reshape(128, KC * 2 * H)
    )
    # [512,32] -> [128, 4*32]
    w3_h = np.ascontiguousarray(
        np.asarray(W3, np.float32).reshape(KC, 128, D).transpose(1, 0, 2).reshape(128, KC * D)
    )
    b1 = np.asarray(b1, np.float32)
    b2 = np.asarray(b2, np.float32)
    bias_h = np.ascontiguousarray(np.concatenate([
        b1[:H].reshape(MJ, 128).T, b1[H:].reshape(MJ, 128).T,
        b2[:H].reshape(MJ, 128).T, b2[H:].reshape(MJ, 128).T,
    ], axis=1))                                                # [128, 16]
    onesb3_h = np.ascontiguousarray(np.concatenate([
        np.ones((1, Bs), np.float32),
        np.asarray(b3, np.float32).reshape(1, D),
    ], axis=1))                                                # [1, Bs+32]

    # ---- build the bass program (same program on all 8 cores) ----
    nc = bacc.Bacc("TRN2", target_bir_lowering=False)

    d_x1i = nc.dram_tensor("x1i", [K1A, Bs], f32r, kind="ExternalInput")  # theta rows + t row
    d_x2i = nc.dram_tensor("x2i", [K1B, Bs], f32r, kind="ExternalInput")  # ctx (static)
    d_th0 = nc.dram_tensor("th0", [D, Bs], f32r, kind="ExternalInput")
    d_w1c1 = nc.dram_tensor("w1c1", [K1A, 2 * H], f32r, kind="ExternalInput")
    d_w1c2 = nc.dram_tensor("w1c2", [K1B, 2 * H], f32r, kind="ExternalInput")
    d_w2 = nc.dram_tensor("w2", [128, KC * 2 * H], f32r, kind="ExternalInput")
    d_w3 = nc.dram_tensor("w3", [128, KC * D], f32r, kind="ExternalInput")
    d_bias = nc.dram_tensor("biases", [128, 4 * MJ], f32, kind="ExternalInput")
    d_ob3 = nc.dram_tensor("onesb3", [1, Bs + D], f32r, kind="ExternalInput")
    d_out = nc.dram_tensor("out", [D, Bs], f32, kind="ExternalOutput")

    # RK4 coefficients: arg scale (for next eval's input), acc scale
    c_arg = [0.5 * dt, 0.5 * dt, dt]
    a_acc = [dt / 6.0, dt / 3.0, dt / 3.0, dt / 6.0]

    with tile.TileContext(nc) as tc:
        PS3_SHARE = int(os.environ.get("KERNEL_PS3SHARE", "0"))
        PSMM_BUFS = 8 if PS3_SHARE else 7
        with (
            tc.tile_pool(name="const", bufs=1) as cpool,
            tc.tile_pool(name="psmm", bufs=PSMM_BUFS, space="PSUM") as ps_pool,
            tc.tile_pool(name="ps3a", bufs=1, space="PSUM") as ps3a_pool,
            tc.tile_pool(name="ps3b", bufs=1, space="PSUM") as ps3b_pool,
            tc.tile_pool(name="sig", bufs=int(os.environ.get("KERNEL_SIGB", "10"))) as sig_pool,
            tc.tile_pool(name="hact", bufs=int(os.environ.get("KERNEL_HB", "20"))) as h_pool,
            tc.tile_pool(name="accp", bufs=int(os.environ.get("KERNEL_AB", "6"))) as acc_pool,
        ):
            tw1c1 = cpool.tile([K1A, 2 * H], f32r)
            tw1c2 = cpool.tile([K1B, 2 * H], f32r)
            tw2 = cpool.tile([128, KC * 2 * H], f32r)
            tw3 = cpool.tile([128, KC * D], f32r)
            tbias = cpool.tile([128, 4 * MJ], f32)
            tb1a = tbias[:, 0 * MJ : 1 * MJ]
            tb1b = tbias[:, 1 * MJ : 2 * MJ]
            tb2a = tbias[:, 2 * MJ : 3 * MJ]
            tb2b = tbias[:, 3 * MJ : 4 * MJ]
            tob3 = cpool.tile([1, Bs + D], f32r)
            tones = tob3[:, 0:Bs]
            tb3 = tob3[:, Bs : Bs + D]
            tx1 = cpool.tile([K1A, Bs], f32r)   # rows: [theta(32) | t(1)]  (dynamic)
            tx2 = cpool.tile([K1B, Bs], f32r)   # ctx (static)
            tth0 = cpool.tile([D, Bs], f32r)    # initial theta

            # layer-1-critical tensors first so eval 0 can start while
            # w2/w3 still stream
            nc.sync.dma_start(tx2[:], d_x2i[:])
            nc.sync.dma_start(tw1c2[:], d_w1c2[:])
            nc.sync.dma_start(tx1[:], d_x1i[:])
            nc.sync.dma_start(tw1c1[:], d_w1c1[:])
            nc.sync.dma_start(tbias[:], d_bias[:])
            nc.sync.dma_start(tth0[:], d_th0[:])
            nc.sync.dma_start(tw2[:], d_w2[:])
            nc.sync.dma_start(tw3[:], d_w3[:])
            nc.sync.dma_start(tob3[:], d_ob3[:])

            def mm(out_ap, lhsT_ap, rhs_ap, start, stop):
                nc.tensor.matmul(out_ap, lhsT_ap, rhs_ap, start=start, stop=stop)

            th_cur = tth0       # theta at start of current step
            t_written = 0.0     # t-row was preloaded with 0

            def issue_l1ctx():
                # static context contribution for the NEXT eval's layer 1 --
                # issued early so PE has work during the RK4 latency chain.
                # One accumulation group per PSUM bank: only the first MM may
                # carry start=True (a second start would zero the whole bank).
                tiles = []
                for j in range(MJ):
                    ps = ps_pool.tile([128, 2 * Bs], f32, tag="psmm")
                    for half, mj in ((1, j + MJ), (0, j)):
                        dst = ps[:, half * Bs : (half + 1) * Bs]
                        msl = slice(mj * 128, (mj + 1) * 128)
                        mm(dst, tw1c2[:, msl], tx2[:],
                           start=(half == 1), stop=False)
                    tiles.append(ps)
                return tiles

            SCHED = int(os.environ.get("KERNEL_SCHED", "3"))
            SPLITP = int(os.environ.get("KERNEL_SPLITPOOLS", "1"))
            SIG1T, SIG2T = ("sig1", "sig2") if SPLITP else ("sig", "sig")
            H1T, H2T = ("h1t", "h2t") if SPLITP else ("hact", "hact")
            if SCHED >= 2 or SCHED == 3:
                ps1 = issue_l1ctx()

            TOFF = (0.0, 0.5, 0.5, 1.0)
            for s in range(steps):
                for e in range(4):
                    TMEMSET_MODE = int(os.environ.get("KERNEL_TMEMSET", "0")) if SCHED == 3 else 0
                    TMEMSET_TAIL = TMEMSET_MODE == 1
                    TMEMSET_POST = TMEMSET_MODE == 2
                    TMEMSET_POOL_TAIL = TMEMSET_MODE == 3
                    t_val = (s + TOFF[e]) * dt
                    if TMEMSET_MODE == 0 and t_val != t_written:
                        nc.gpsimd.memset(tx1[D : D + 1, :].bitcast(f32), float(t_val))
                        t_written = t_val

                    last_eval = (s == steps - 1) and (e == 3)

                    if SCHED == 3:
                        # v1.5 structure, ctx MMs pre-issued (ps1 tiles);
                        # group stop goes on the LAST MM of the bank (a-half)
                        h1 = []
                        for j in range(MJ):
                            ps = ps1[j]
                            for half, mj in ((1, j + MJ), (0, j)):
                                dst = ps[:, half * Bs : (half + 1) * Bs]
                                msl = slice(mj * 128, (mj + 1) * 128)
                                mm(dst, tw1c1[:, msl], tx1[:], start=False,
                                   stop=(half == 0))
                            sg = sig_pool.tile([128, Bs], f32, tag=SIG1T)
                            nc.scalar.activation(
                                sg[:], ps[:, Bs : 2 * Bs], SIGMOID,
                                bias=tb1b[:, j : j + 1]
                            )
                            ht = h_pool.tile([128, Bs], f32r, tag=H1T)
                            nc.vector.scalar_tensor_tensor(
                                ht[:], ps[:, 0:Bs], tb1a[:, j : j + 1], sg[:],
                                ALU.add, ALU.mult,
                            )
                            h1.append(ht)
                        h2 = []
                        for j in range(MJ):
                            ps = ps_pool.tile([128, 2 * Bs], f32, tag="psmm")
                            # b-half group first so the sigmoid overlaps the
                            # a-half matmuls (keeps ACT off the eval tail)
                            dstb = ps[:, Bs : 2 * Bs]
                            for kc in range(KC):
                                csl = slice(kc * 2 * H + (j + MJ) * 128,
                                            kc * 2 * H + (j + MJ + 1) * 128)
                                mm(dstb, tw2[:, csl], h1[kc][:],
                                   start=(kc == 0), stop=(kc == KC - 1))
                            sg = sig_pool.tile([128, Bs], f32, tag=SIG2T)
                            nc.scalar.activation(
                                sg[:], dstb, SIGMOID, bias=tb2b[:, j : j + 1]
                            )
                            dsta = ps[:, 0:Bs]
                            for kc in range(KC):
                                csl = slice(kc * 2 * H + j * 128,
                                            kc * 2 * H + (j + 1) * 128)
                                mm(dsta, tw2[:, csl], h1[kc][:],
                                   start=(kc == 0), stop=(kc == KC - 1))
                            ht = h_pool.tile([128, Bs], f32r, tag=H2T)
                            nc.vector.scalar_tensor_tensor(
                                ht[:], dsta, tb2a[:, j : j + 1], sg[:],
                                ALU.add, ALU.mult,
                            )
                            h2.append(ht)
                        if not last_eval:
                            ps1_next = issue_l1ctx()
                    elif SCHED == 0:
                        # v1.5: per-j, both halves, GLU immediately
                        h1 = []
                        for j in range(MJ):
                            ps = ps_pool.tile([128, 2 * Bs], f32, tag="psmm")
                            for half, mj in ((0, j), (1, j + MJ)):
                                dst = ps[:, half * Bs : (half + 1) * Bs]
                                msl = slice(mj * 128, (mj + 1) * 128)
                                mm(dst, tw1c2[:, msl], tx2[:], start=True, stop=False)
                                mm(dst, tw1c1[:, msl], tx1[:], start=False, stop=True)
                            sg = sig_pool.tile([128, Bs], f32, tag="sig")
                            nc.scalar.activation(
                                sg[:], ps[:, Bs : 2 * Bs], SIGMOID,
                                bias=tb1b[:, j : j + 1]
                            )
                            ht = h_pool.tile([128, Bs], f32r, tag="hact")
                            nc.vector.scalar_tensor_tensor(
                                ht[:], ps[:, 0:Bs], tb1a[:, j : j + 1], sg[:],
                                ALU.add, ALU.mult,
                            )
                            h1.append(ht)
                        h2 = []
                        for j in range(MJ):
                            ps = ps_pool.tile([128, 2 * Bs], f32, tag="psmm")
                            for half, mj in ((0, j), (1, j + MJ)):
                                dst = ps[:, half * Bs : (half + 1) * Bs]
                                for kc in range(KC):
                                    csl = slice(kc * 2 * H + mj * 128,
                                                kc * 2 * H + (mj + 1) * 128)
                                    mm(dst, tw2[:, csl], h1[kc][:],
                                       start=(kc == 0), stop=(kc == KC - 1))
                            sg = sig_pool.tile([128, Bs], f32, tag="sig")
                            nc.scalar.activation(
                                sg[:], ps[:, Bs : 2 * Bs], SIGMOID,
                                bias=tb2b[:, j : j + 1]
                            )
                            ht = h_pool.tile([128, Bs], f32r, tag="hact")
                            nc.vector.scalar_tensor_tensor(
                                ht[:], ps[:, 0:Bs], tb2a[:, j : j + 1], sg[:],
                                ALU.add, ALU.mult,
                            )
                            h2.append(ht)
                    else:
                        if SCHED < 2:
                            ps1 = issue_l1ctx()
                        sg1 = []
                        for j in range(MJ):
                            dst = ps1[j][:, Bs : 2 * Bs]
                            msl = slice((j + MJ) * 128, (j + MJ + 1) * 128)
                            mm(dst, tw1c1[:, msl], tx1[:], start=False, stop=True)
                            sg = sig_pool.tile([128, Bs], f32, tag="sig")
                            nc.scalar.activation(
                                sg[:], dst, SIGMOID, bias=tb1b[:, j : j + 1]
                            )
                            sg1.append(sg)
                        h1 = []
                        for j in range(MJ):
                            dst = ps1[j][:, 0:Bs]
                            msl = slice(j * 128, (j + 1) * 128)
                            mm(dst, tw1c1[:, msl], tx1[:], start=False, stop=True)
                            ht = h_pool.tile([128, Bs], f32r, tag="hact")
                            nc.vector.scalar_tensor_tensor(
                                ht[:], dst, tb1a[:, j : j + 1], sg1[j][:],
                                ALU.add, ALU.mult,
                            )
                            h1.append(ht)
                        ps2 = []
                        for j in range(MJ):
                            ps2j = ps_pool.tile([128, 2 * Bs], f32, tag="psmm")
                            ps2.append(ps2j)
                        for kc in range(KC - 1):
                            for j in range(MJ):
                                csl = slice(kc * 2 * H + (j + MJ) * 128,
                                            kc * 2 * H + (j + MJ + 1) * 128)
                                mm(ps2[j][:, Bs : 2 * Bs], tw2[:, csl], h1[kc][:],
                                   start=(kc == 0), stop=False)
                            for j in range(MJ):
                                csl = slice(kc * 2 * H + j * 128,
                                            kc * 2 * H + (j + 1) * 128)
                                mm(ps2[j][:, 0:Bs], tw2[:, csl], h1[kc][:],
                                   start=(kc == 0), stop=False)
                        kc = KC - 1
                        sg2 = []
                        for j in range(MJ):
                            csl = slice(kc * 2 * H + (j + MJ) * 128,
                                        kc * 2 * H + (j + MJ + 1) * 128)
                            dst = ps2[j][:, Bs : 2 * Bs]
                            mm(dst, tw2[:, csl], h1[kc][:], start=False, stop=True)
                            sg = sig_pool.tile([128, Bs], f32, tag="sig")
                            nc.scalar.activation(
                                sg[:], dst, SIGMOID, bias=tb2b[:, j : j + 1]
                            )
                            sg2.append(sg)
                        h2 = []
                        for j in range(MJ):
                            csl = slice(kc * 2 * H + j * 128,
                                        kc * 2 * H + (j + 1) * 128)
                            dst = ps2[j][:, 0:Bs]
                            mm(dst, tw2[:, csl], h1[kc][:], start=False, stop=True)
                            ht = h_pool.tile([128, Bs], f32r, tag="hact")
                            nc.vector.scalar_tensor_tensor(
                                ht[:], dst, tb2a[:, j : j + 1], sg2[j][:],
                                ALU.add, ALU.mult,
                            )
                            h2.append(ht)
                        if SCHED >= 2 and not last_eval:
                            ps1_next = issue_l1ctx()

                    # ---- layer 3: k = h2 @ W3 (+ b3) in PSUM ----
                    if PS3_SHARE:
                        ps3full = ps_pool.tile([128, 2 * Bs], f32, tag="psmm")
                        ps3 = ps3full[0:D, 0:Bs]
                    else:
                        ps3 = ps3_pool.tile([D, Bs], f32, tag="ps3")
                    for kc in range(KC):
                        mm(ps3[:], tw3[:, kc * D : (kc + 1) * D], h2[kc][:],
                           start=(kc == 0), stop=(kc == KC - 1 and b3_is_zero))
                    if not b3_is_zero:
                        mm(ps3[:], tb3[:], tones[:], start=False, stop=True)

                    # ---- RK4 bookkeeping ----
                    if TMEMSET_POOL_TAIL and not last_eval:
                        nxt_s, nxt_e = (s, e + 1) if e < 3 else (s + 1, 0)
                        nxt_t = (nxt_s + TOFF[nxt_e]) * dt
                        if nxt_t != t_written:
                            nc.gpsimd.memset(
                                tx1[D : D + 1, :].bitcast(f32), float(nxt_t)
                            )
                            t_written = nxt_t
                    if TMEMSET_TAIL and not last_eval:
                        # write the NEXT eval's t-row on DVE (same engine as
                        # the arg STT -> no extra cross-engine hop on the
                        # arg -> layer-1 chain)
                        nxt_s, nxt_e = (s, e + 1) if e < 3 else (s + 1, 0)
                        nxt_t = (nxt_s + TOFF[nxt_e]) * dt
                        if nxt_t != t_written:
                            nc.vector.memset(
                                tx1[D : D + 1, :].bitcast(f32), float(nxt_t)
                            )
                            t_written = nxt_t
                    if e < 3:
                        nc.vector.scalar_tensor_tensor(
                            tx1[0:D, :], ps3[:], float(c_arg[e]), th_cur[:],
                            ALU.mult, ALU.add,
                        )
                    if TMEMSET_POST and not last_eval:
                        nxt_s, nxt_e = (s, e + 1) if e < 3 else (s + 1, 0)
                        nxt_t = (nxt_s + TOFF[nxt_e]) * dt
                        if nxt_t != t_written:
                            nc.vector.memset(
                                tx1[D : D + 1, :].bitcast(f32), float(nxt_t)
                            )
                            t_written = nxt_t
                    base = th_cur if e == 0 else acc_prev
                    if e == 3 and s != steps - 1:
                        # theta_{s+1} goes straight into the matmul input tile
                        # (keeps the Pool copy off the critical chain)...
                        nc.vector.scalar_tensor_tensor(
                            tx1[0:D, :], ps3[:], float(a_acc[e]), base[:],
                            ALU.mult, ALU.add,
                        )
                    acc_new = acc_pool.tile([D, Bs], f32, tag="accp")
                    # ...and also into its own tile (used as th_cur next step)
                    nc.vector.scalar_tensor_tensor(
                        acc_new[:], ps3[:], float(a_acc[e]), base[:],
                        ALU.mult, ALU.add,
                    )
                    acc_prev = acc_new
                    if SCHED >= 2 and not last_eval:
                        ps1 = ps1_next

                th_cur = acc_prev  # theta_{s+1}

            nc.sync.dma_start(d_out[:], th_cur[:])

    # ---- per-core input maps ----
    in_maps = []
    for c in range(N_CORES):
        sl = slice(c * Bs, (c + 1) * Bs)
        th_T = np.ascontiguousarray(np.asarray(theta0[sl], np.float32).T)
        ctx_T = np.ascontiguousarray(np.asarray(context[sl], np.float32).T)
        x1i = np.concatenate([th_T, np.zeros((1, Bs), np.float32)], axis=0)
        in_maps.append(
            {
                "x1i": np.ascontiguousarray(x1i),
                "x2i": ctx_T,
                "th0": th_T,
                "w1c1": w1c1_h,
                "w1c2": w1c2_h,
                "w2": w2_h,
                "w3": w3_h,
                "biases": bias_h,
                "onesb3": onesb3_h,
            }
        )

    return nc, in_maps


def _build_and_run(theta0, context, W1, b1, W2, b2, W3, b3, n_steps):
    from concourse.bass_utils import run_bass_kernel_spmd

    nc, in_maps = _build_program(theta0, context, W1, b1, W2, b2, W3, b3, n_steps)
    nc.finalize()  # Bacc: split multi-sem waits + allocate registers
    res = run_bass_kernel_spmd(
        nc,
        in_maps,
        core_ids=list(range(N_CORES)),
        trace=bool(int(os.environ.get("KERNEL_TRACE", "0"))),
    )
    _build_and_run.last_results = res

    out = np.concatenate([r["out"].T for r in res.results], axis=0)
    return np.ascontiguousarray(out.astype(np.float32))


def kernel(theta0, context, W1, b1, W2, b2, W3, b3, n_steps):
    return _build_and_run(
        np.asarray(theta0), np.asarray(context), W1, b1, W2, b2, W3, b3, n_steps
    )



# revision 26
# speedup vs baseline: 1.3299x; 1.0019x over previous
"""CCNF RK4 sampling kernel for 8 Trainium2 NeuronCores.

Data-parallel: batch 2048 -> 256 per core, weights replicated.
On-core layout: features on partitions, batch on the free dim (N=256).
Matmuls run in float32r (fp32 data, fast PE mode).
"""

import os

import numpy as np

N_CORES = 8


def _build_program_fast(theta0, context, W1, b1, W2, b2, W3, b3, n_steps):
    """Two-stream fast path (requires zero biases).

    Per core: batch 256 split into two independent 128-sample streams so
    each stream's sigmoid/GLU chain overlaps the other stream's matmuls.
    Layer 1 and 3 run in bf16, layer 2 in fp8e4m3 with DoubleRow (2x PE).
    """
    import ml_dtypes

    import concourse.mybir as mybir
    import concourse.tile as tile
    from concourse import bacc

    f32 = mybir.dt.float32
    bf16 = mybir.dt.bfloat16
    f8 = mybir.dt.float8e4
    ALU = mybir.AluOpType
    SIGMOID = mybir.ActivationFunctionType.Sigmoid
    DR = mybir.MatmulPerfMode.DoubleRow

    np_bf16 = ml_dtypes.bfloat16
    np_f8 = ml_dtypes.float8_e4m3

    B, D = theta0.shape          # 2048, 32
    C = context.shape[1]         # 128
    IN, H2 = W1.shape            # 161, 1024
    H = W2.shape[0]              # 512
    assert H2 == 2 * H and W2.shape[1] == 2 * H and W3.shape == (H, D)
    assert IN == D + 1 + C and D == 32 and C == 128 and H == 512
    assert B % (N_CORES * 2) == 0
    Bs = B // N_CORES            # 256 per core
    Ns = Bs // 2                 # 128 per stream
    steps = int(n_steps)
    dtv = 1.0 / steps

    S_H1 = 16.0                  # h1 tiles carry 16*h1 in fp8
    S_W2 = 32.0                  # W2 stored as 32*W2 in fp8

    # ---- host-side packing (shared across cores) ----
    W1 = np.asarray(W1, np.float32)
    w1a_h = np.ascontiguousarray(W1[: D + 1]).astype(np_bf16)      # [33,1024]
    w1c_h = np.ascontiguousarray(W1[D + 1 :]).astype(np_bf16)      # [128,1024]
    # W2 -> DoubleRow pairs: w2p[p, i, g*2H + m] = 32*W2[g*256 + i*128 + p, m]
    w2 = np.asarray(W2, np.float32) * S_W2
    w2p_h = np.ascontiguousarray(
        w2.reshape(2, 2, 128, 2 * H).transpose(2, 1, 0, 3).reshape(128, 4 * 2 * H)
    ).astype(np_f8)                                                # [128, 8192]
    # W3 -> [128, KC*D]: cols kc*D+d = W3[kc*128+p, d]
    w3_h = np.ascontiguousarray(
        np.asarray(W3, np.float32).reshape(4, 128, D).transpose(1, 0, 2).reshape(128, 4 * D)
    ).astype(np_bf16)

    c_arg = [0.5 * dtv, 0.5 * dtv, dtv]
    a_acc = [dtv / 6.0, dtv / 3.0, dtv / 3.0, dtv / 6.0]
    TOFF = (0.0, 0.5, 0.5, 1.0)

    nc = bacc.Bacc("TRN2", target_bir_lowering=False)

    d_x1 = nc.dram_tensor("x1", [D + 1, Bs], bf16, kind="ExternalInput")
    d_ctx = nc.dram_tensor("ctx", [C, Bs], bf16, kind="ExternalInput")
    d_th0 = nc.dram_tensor("th0", [D, Bs], f32, kind="ExternalInput")
    d_w1a = nc.dram_tensor("w1a", [D + 1, 2 * H], bf16, kind="ExternalInput")
    d_w1c = nc.dram_tensor("w1c", [C, 2 * H], bf16, kind="ExternalInput")
    d_w2p = nc.dram_tensor("w2p", [128, 4 * 2 * H], f8, kind="ExternalInput")
    d_w3 = nc.dram_tensor("w3", [128, 4 * D], bf16, kind="ExternalInput")
    d_out = nc.dram_tensor("out", [D, Bs], f32, kind="ExternalOutput")

    with tile.TileContext(nc) as tc:
        
        with (
            tc.tile_pool(name="const", bufs=1) as cpool,
            tc.tile_pool(name="psl1", bufs=3, space="PSUM") as ps1_pool,
            tc.tile_pool(name="psl2", bufs=3, space="PSUM") as ps2_pool,
            tc.tile_pool(name="ps3a", bufs=1, space="PSUM") as ps3a_pool,
            tc.tile_pool(name="ps3b", bufs=1, space="PSUM") as ps3b_pool,
            tc.tile_pool(name="sig", bufs=int(os.environ.get("KERNEL_SIGB", "8"))) as sig_pool,
            tc.tile_pool(name="hact", bufs=int(os.environ.get("KERNEL_HB", "12"))) as h_pool,
            tc.tile_pool(name="accp", bufs=int(os.environ.get("KERNEL_AB", "8"))) as acc_pool,
        ):
            tw1a = cpool.tile([D + 1, 2 * H], bf16)
            tw1c = cpool.tile([C, 2 * H], bf16)
            tw2p = cpool.tile([128, 2, 2 * 2 * H], f8)  # [p, i, g*2H+m]
            tw3 = cpool.tile([128, 4 * D], bf16)
            tctx = cpool.tile([C, Bs], bf16)            # both streams
            txp = [[cpool.tile([D + 1, Ns], bf16, name=f"txp{u}_{p}") for p in range(2)]
                   for u in range(2)]
            tth0 = cpool.tile([D, Bs], f32)
            twz = cpool.tile([1, 128], bf16)
            tout = cpool.tile([D, Bs], f32)

            # warmup: burn the PE p-state ramp while input DMAs stream
            nc.gpsimd.memset(twz[:], 0.0)
            NWARM = int(os.environ.get("KERNEL_WARM", "40"))
            if NWARM:
                psw = ps1_pool.tile([128, 2 * Bs], f32, tag="psmm", name="psw")
                for i in range(NWARM):
                    nc.tensor.matmul(psw[:, 0:128], twz[:], twz[:],
                                     start=(i == 0), stop=(i == NWARM - 1))

            # l1-critical tensors first (dispatch order == arrival order)
            nc.sync.dma_start(txp[0][0][:], d_x1[:, 0:Ns])
            nc.sync.dma_start(txp[1][0][:], d_x1[:, Ns:Bs])
            nc.sync.dma_start(tctx[:], d_ctx[:])
            nc.sync.dma_start(tw1c[:], d_w1c[:])
            nc.sync.dma_start(tw1a[:], d_w1a[:])
            nc.sync.dma_start(tw2p[:], d_w2p[:])
            nc.sync.dma_start(tw3[:], d_w3[:])
            nc.sync.dma_start(tth0[:], d_th0[:])

            def mm(out_ap, lhsT_ap, rhs_ap, start, stop, pm=None):
                nc.tensor.matmul(out_ap, lhsT_ap, rhs_ap, start=start,
                                 stop=stop, perf_mode=pm)

            # W1 column base per bank: A01, A23, B01, B23
            L1_BANKS = ((0, "A01"), (256, "A23"), (512, "B01"), (768, "B23"))

            th_cur = [tth0[:, 0:Ns], tth0[:, Ns:Bs]]
            acc_prev = [None, None]

            def alloc_l1_banks():
                bb, ba = [], []
                for u in range(2):
                    tb = ps1_pool.tile([128, 2 * Bs], f32, tag="psmm", name=f"l1B{u}")
                    ta = ps1_pool.tile([128, 2 * Bs], f32, tag="psmm", name=f"l1A{u}")
                    bb.append(tb)
                    ba.append(ta)
                return bb, ba

            def emit_ctx(bb, ba):
                # ctx half of layer 1 for the NEXT eval: tx-independent,
                # fills PE idle during the current eval's tail
                for u in range(2):
                    ctxu = tctx[:, u * Ns : (u + 1) * Ns]
                    for jb in range(4):
                        mm(bb[u][:, jb * 128 : (jb + 1) * 128],
                           tw1c[:, H + jb * 128 : H + (jb + 1) * 128],
                           ctxu, start=(jb == 0), stop=False)
                    for ja in range(4):
                        mm(ba[u][:, ja * 128 : (ja + 1) * 128],
                           tw1c[:, ja * 128 : (ja + 1) * 128],
                           ctxu, start=(ja == 0), stop=False)

            nbanks = alloc_l1_banks()
            emit_ctx(*nbanks)

            for s in range(steps):
                for e in range(4):
                    last_eval = (s == steps - 1) and (e == 3)
                    par = (4 * s + e) % 2
                    bankB, bankA = nbanks

                    # ---- layer 1 theta MMs + GLU per stream
                    hp = [None, None]
                    for u in range(2):
                        for jb in range(4):
                            mm(bankB[u][:, jb * 128 : (jb + 1) * 128],
                               tw1a[:, H + jb * 128 : H + (jb + 1) * 128],
                               txp[u][par][:], start=False, stop=(jb == 3))
                        sg1 = sig_pool.tile([128, 2 * Bs], f32, tag=f"sg{u}", name=f"sg1_{u}")
                        nc.scalar.activation(sg1[:, 0:Bs], bankB[u][:, 0:Bs], SIGMOID)
                        nc.scalar.activation(sg1[:, Bs : 2 * Bs],
                                             bankB[u][:, Bs : 2 * Bs], SIGMOID)
                        for ja in range(4):
                            mm(bankA[u][:, ja * 128 : (ja + 1) * 128],
                               tw1a[:, ja * 128 : (ja + 1) * 128],
                               txp[u][par][:], start=False, stop=(ja == 3))
                        ht = h_pool.tile([128, 4, 128], f8, tag=f"hp{u}", name=f"hp{u}")
                        nc.vector.scalar_tensor_tensor(
                            ht[:, 0:2, :], bankA[u][:, 0:Bs], S_H1,
                            sg1[:, 0:Bs], ALU.mult, ALU.mult,
                        )
                        nc.vector.scalar_tensor_tensor(
                            ht[:, 2:4, :], bankA[u][:, Bs : 2 * Bs], S_H1,
                            sg1[:, Bs : 2 * Bs], ALU.mult, ALU.mult,
                        )
                        hp[u] = ht

                    # ---- layer 2 per stream
                    hq = [None, None]
                    bases = [th_cur[u] if e == 0 else acc_prev[u] for u in range(2)]
                    for u in range(2):
                        b2B = ps2_pool.tile([128, 2 * Bs], f32, tag="psmm", name=f"l2B{u}")
                        b2A = ps2_pool.tile([128, 2 * Bs], f32, tag="psmm", name=f"l2A{u}")
                        for g in range(2):
                            for jb in range(4):
                                mj = 4 + jb
                                csl = slice(g * 2 * H + mj * 128, g * 2 * H + (mj + 1) * 128)
                                mm(b2B[:, jb * 128 : (jb + 1) * 128],
                                   tw2p[:, :, csl], hp[u][:, 2 * g : 2 * g + 2, :],
                                   start=(g == 0 and jb == 0), stop=(g == 1 and jb == 3),
                                   pm=DR)
                        sg2 = sig_pool.tile([128, 2 * Bs], f32, tag=f"s2{u}", name=f"sg2_{u}")
                        nc.scalar.activation(
                            sg2[:, 0:Bs], b2B[:, 0:Bs], SIGMOID,
                            scale=1.0 / (S_H1 * S_W2),
                        )
                        nc.scalar.activation(
                            sg2[:, Bs : 2 * Bs], b2B[:, Bs : 2 * Bs], SIGMOID,
                            scale=1.0 / (S_H1 * S_W2),
                        )
                        for g in range(2):
                            for ja in range(4):
                                csl = slice(g * 2 * H + ja * 128, g * 2 * H + (ja + 1) * 128)
                                mm(b2A[:, ja * 128 : (ja + 1) * 128],
                                   tw2p[:, :, csl], hp[u][:, 2 * g : 2 * g + 2, :],
                                   start=(g == 0 and ja == 0), stop=(g == 1 and ja == 3),
                                   pm=DR)
                        ht = h_pool.tile([128, 4, 128], bf16, tag=f"hq{u}", name=f"hq{u}")
                        nc.vector.scalar_tensor_tensor(
                            ht[:, 0:2, :], b2A[:, 0:Bs],
                            1.0 / (S_H1 * S_W2), sg2[:, 0:Bs],
                            ALU.mult, ALU.mult,
                        )
                        nc.vector.scalar_tensor_tensor(
                            ht[:, 2:4, :], b2A[:, Bs : 2 * Bs],
                            1.0 / (S_H1 * S_W2), sg2[:, Bs : 2 * Bs],
                            ALU.mult, ALU.mult,
                        )
                        hq[u] = ht

                    # ---- ctx MMs of the NEXT eval (PE filler during tail)
                    if not last_eval:
                        nbanks = alloc_l1_banks()
                        emit_ctx(*nbanks)

                    # ---- layer 3 + tx per stream
                    ps3s = [None, None]
                    for u in range(2):
                        ps3s[u] = (ps3a_pool if u == 0 else ps3b_pool).tile(
                            [D, Ns], f32, tag=f"ps3{u}", name=f"ps3{u}")
                        for kc in range(4):
                            mm(ps3s[u][:],
                               tw3[:, kc * D : (kc + 1) * D],
                               hq[u][:, kc : kc + 1, :],
                               start=(kc == 0), stop=(kc == 3))
                        nxt = txp[u][1 - par]
                        if e < 3:
                            nc.vector.scalar_tensor_tensor(
                                nxt[0:D, :], ps3s[u][:], float(c_arg[e]), th_cur[u],
                                ALU.mult, ALU.add,
                            )
                        elif not last_eval:
                            nc.vector.scalar_tensor_tensor(
                                nxt[0:D, :], ps3s[u][:], float(a_acc[e]), bases[u],
                                ALU.mult, ALU.add,
                            )
                        if not last_eval:
                            nxt_s, nxt_e = (s, e + 1) if e < 3 else (s + 1, 0)
                            nxt_t = (nxt_s + TOFF[nxt_e]) * dtv
                            nc.gpsimd.memset(nxt[D : D + 1, :], float(nxt_t))

                    for u in range(2):
                        if last_eval:
                            acc_new = tout[:, u * Ns : (u + 1) * Ns]
                        else:
                            acc_new = acc_pool.tile(
                                [D, Ns], f32, tag=f"acc{u}", name=f"acc{u}")[:]
                        nc.vector.scalar_tensor_tensor(
                            acc_new, ps3s[u][:], float(a_acc[e]), bases[u],
                            ALU.mult, ALU.add,
                        )
                        acc_prev[u] = acc_new

                for u in range(2):
                    th_cur[u] = acc_prev[u]

            nc.sync.dma_start(d_out[:], tout[:])

    # ---- per-core input maps ----
    in_maps = []
    for c in range(N_CORES):
        sl = slice(c * Bs, (c + 1) * Bs)
        th_T = np.ascontiguousarray(np.asarray(theta0[sl], np.float32).T)
        ctx_T = np.ascontiguousarray(np.asarray(context[sl], np.float32).T)
        x1 = np.concatenate([th_T, np.zeros((1, Bs), np.float32)], axis=0)
        in_maps.append(
            {
                "x1": np.ascontiguousarray(x1.astype(np_bf16)),
                "ctx": np.ascontiguousarray(ctx_T.astype(np_bf16)),
                "th0": th_T,
                "w1a": w1a_h,
                "w1c": w1c_h,
                "w2p": w2p_h,
                "w3": w3_h,
            }
        )
    return nc, in_maps


def _build_program(theta0, context, W1, b1, W2, b2, W3, b3, n_steps):
    if (
        int(os.environ.get("KERNEL_FAST", "1"))
        and not np.any(np.asarray(b1))
        and not np.any(np.asarray(b2))
        and not np.any(np.asarray(b3))
        and theta0.shape == (2048, 32)
        and int(n_steps) == 16
    ):
        return _build_program_fast(
            theta0, context, W1, b1, W2, b2, W3, b3, n_steps
        )
    return _build_program_v1(theta0, context, W1, b1, W2, b2, W3, b3, n_steps)


def _build_program_v1(theta0, context, W1, b1, W2, b2, W3, b3, n_steps):
    import concourse.bass as bass
    import concourse.mybir as mybir
    import concourse.tile as tile
    from concourse import bacc
    from concourse.bass_utils import run_bass_kernel_spmd

    f32 = mybir.dt.float32
    f32r = mybir.dt.float32r
    ALU = mybir.AluOpType
    SIGMOID = mybir.ActivationFunctionType.Sigmoid

    B, D = theta0.shape          # 2048, 32
    C = context.shape[1]         # 128
    IN, H2 = W1.shape            # 161, 1024
    H = W2.shape[0]              # 512
    assert H2 == 2 * H and W2.shape[1] == 2 * H and W3.shape == (H, D)
    assert IN == D + 1 + C
    assert B % N_CORES == 0
    Bs = B // N_CORES            # 256 per core
    steps = int(n_steps)
    dt = 1.0 / steps

    KC = H // 128                # 4 k-chunks for layer 2/3
    MJ = H // 128                # 4 output column-chunks per GLU half
    # layer-1 K split: rows [0:33) = theta(32)+t(1) (dynamic); rows [33:161) = ctx (static)
    K1A = D + 1                  # 33 (theta + t)
    K1B = IN - K1A               # 128 (ctx)

    # ---- host-side layout prep (shared across cores) ----
    W1 = np.ascontiguousarray(W1, np.float32)
    w1c1_h = np.ascontiguousarray(W1[:K1A])                    # [33, 1024]  theta+t rows
    w1c2_h = np.ascontiguousarray(W1[K1A:])                    # [128, 1024] ctx rows
    b3_is_zero = not np.any(np.asarray(b3, np.float32))
    # [512,1024] -> [128, 4*1024]: chunk kc at cols kc*1024
    w2_h = np.ascontiguousarray(
        np.asarray(W2, np.float32).reshape(KC, 128, 2 * H).transpose(1, 0, 2).reshape(128, KC * 2 * H)
    )
    # [512,32] -> [128, 4*32]
    w3_h = np.ascontiguousarray(
        np.asarray(W3, np.float32).reshape(KC, 128, D).transpose(1, 0, 2).reshape(128, KC * D)
    )
    b1 = np.asarray(b1, np.float32)
    b2 = np.asarray(b2, np.float32)
    bias_h = np.ascontiguousarray(np.concatenate([
        b1[:H].reshape(MJ, 128).T, b1[H:].reshape(MJ, 128).T,
        b2[:H].reshape(MJ, 128).T, b2[H:].reshape(MJ, 128).T,
    ], axis=1))                                                # [128, 16]
    onesb3_h = np.ascontiguousarray(np.concatenate([
        np.ones((1, Bs), np.float32),
        np.asarray(b3, np.float32).reshape(1, D),
    ], axis=1))                                                # [1, Bs+32]

    # ---- build the bass program (same program on all 8 cores) ----
    nc = bacc.Bacc("TRN2", target_bir_lowering=False)

    d_x1i = nc.dram_tensor("x1i", [K1A, Bs], f32r, kind="ExternalInput")  # theta rows + t row
    d_x2i = nc.dram_tensor("x2i", [K1B, Bs], f32r, kind="ExternalInput")  # ctx (static)
    d_th0 = nc.dram_tensor("th0", [D, Bs], f32r, kind="ExternalInput")
    d_w1c1 = nc.dram_tensor("w1c1", [K1A, 2 * H], f32r, kind="ExternalInput")
    d_w1c2 = nc.dram_tensor("w1c2", [K1B, 2 * H], f32r, kind="ExternalInput")
    d_w2 = nc.dram_tensor("w2", [128, KC * 2 * H], f32r, kind="ExternalInput")
    d_w3 = nc.dram_tensor("w3", [128, KC * D], f32r, kind="ExternalInput")
    d_bias = nc.dram_tensor("biases", [128, 4 * MJ], f32, kind="ExternalInput")
    d_ob3 = nc.dram_tensor("onesb3", [1, Bs + D], f32r, kind="ExternalInput")
    d_out = nc.dram_tensor("out", [D, Bs], f32, kind="ExternalOutput")

    # RK4 coefficients: arg scale (for next eval's input), acc scale
    c_arg = [0.5 * dt, 0.5 * dt, dt]
    a_acc = [dt / 6.0, dt / 3.0, dt / 3.0, dt / 6.0]

    with tile.TileContext(nc) as tc:
        PS3_SHARE = int(os.environ.get("KERNEL_PS3SHARE", "0"))
        PSMM_BUFS = 8 if PS3_SHARE else 7
        with (
            tc.tile_pool(name="const", bufs=1) as cpool,
            tc.tile_pool(name="psmm", bufs=PSMM_BUFS, space="PSUM") as ps_pool,
            tc.tile_pool(name="ps3a", bufs=1, space="PSUM") as ps3a_pool,
            tc.tile_pool(name="ps3b", bufs=1, space="PSUM") as ps3b_pool,
            tc.tile_pool(name="sig", bufs=int(os.environ.get("KERNEL_SIGB", "10"))) as sig_pool,
            tc.tile_pool(name="hact", bufs=int(os.environ.get("KERNEL_HB", "20"))) as h_pool,
            tc.tile_pool(name="accp", bufs=int(os.environ.get("KERNEL_AB", "6"))) as acc_pool,
        ):
            tw1c1 = cpool.tile([K1A, 2 * H], f32r)
            tw1c2 = cpool.tile([K1B, 2 * H], f32r)
            tw2 = cpool.tile([128, KC * 2 * H], f32r)
            tw3 = cpool.tile([128, KC * D], f32r)
            tbias = cpool.tile([128, 4 * MJ], f32)
            tb1a = tbias[:, 0 * MJ : 1 * MJ]
            tb1b = tbias[:, 1 * MJ : 2 * MJ]
            tb2a = tbias[:, 2 * MJ : 3 * MJ]
            tb2b = tbias[:, 3 * MJ : 4 * MJ]
            tob3 = cpool.tile([1, Bs + D], f32r)
            tones = tob3[:, 0:Bs]
            tb3 = tob3[:, Bs : Bs + D]
            tx1 = cpool.tile([K1A, Bs], f32r)   # rows: [theta(32) | t(1)]  (dynamic)
            tx2 = cpool.tile([K1B, Bs], f32r)   # ctx (static)
            tth0 = cpool.tile([D, Bs], f32r)    # initial theta

            # layer-1-critical tensors first so eval 0 can start while
            # w2/w3 still stream
            nc.sync.dma_start(tx2[:], d_x2i[:])
            nc.sync.dma_start(tw1c2[:], d_w1c2[:])
            nc.sync.dma_start(tx1[:], d_x1i[:])
            nc.sync.dma_start(tw1c1[:], d_w1c1[:])
            nc.sync.dma_start(tbias[:], d_bias[:])
            nc.sync.dma_start(tth0[:], d_th0[:])
            nc.sync.dma_start(tw2[:], d_w2[:])
            nc.sync.dma_start(tw3[:], d_w3[:])
            nc.sync.dma_start(tob3[:], d_ob3[:])

            def mm(out_ap, lhsT_ap, rhs_ap, start, stop):
                nc.tensor.matmul(out_ap, lhsT_ap, rhs_ap, start=start, stop=stop)

            th_cur = tth0       # theta at start of current step
            t_written = 0.0     # t-row was preloaded with 0

            def issue_l1ctx():
                # static context contribution for the NEXT eval's layer 1 --
                # issued early so PE has work during the RK4 latency chain.
                # One accumulation group per PSUM bank: only the first MM may
                # carry start=True (a second start would zero the whole bank).
                tiles = []
                for j in range(MJ):
                    ps = ps_pool.tile([128, 2 * Bs], f32, tag="psmm")
                    for half, mj in ((1, j + MJ), (0, j)):
                        dst = ps[:, half * Bs : (half + 1) * Bs]
                        msl = slice(mj * 128, (mj + 1) * 128)
                        mm(dst, tw1c2[:, msl], tx2[:],
                           start=(half == 1), stop=False)
                    tiles.append(ps)
                return tiles

            SCHED = int(os.environ.get("KERNEL_SCHED", "3"))
            SPLITP = int(os.environ.get("KERNEL_SPLITPOOLS", "1"))
            SIG1T, SIG2T = ("sig1", "sig2") if SPLITP else ("sig", "sig")
            H1T, H2T = ("h1t", "h2t") if SPLITP else ("hact", "hact")
            if SCHED >= 2 or SCHED == 3:
                ps1 = issue_l1ctx()

            TOFF = (0.0, 0.5, 0.5, 1.0)
            for s in range(steps):
                for e in range(4):
                    TMEMSET_MODE = int(os.environ.get("KERNEL_TMEMSET", "0")) if SCHED == 3 else 0
                    TMEMSET_TAIL = TMEMSET_MODE == 1
                    TMEMSET_POST = TMEMSET_MODE == 2
                    TMEMSET_POOL_TAIL = TMEMSET_MODE == 3
                    t_val = (s + TOFF[e]) * dt
                    if TMEMSET_MODE == 0 and t_val != t_written:
                        nc.gpsimd.memset(tx1[D : D + 1, :].bitcast(f32), float(t_val))
                        t_written = t_val

                    last_eval = (s == steps - 1) and (e == 3)

                    if SCHED == 3:
                        # v1.5 structure, ctx MMs pre-issued (ps1 tiles);
                        # group stop goes on the LAST MM of the bank (a-half)
                        h1 = []
                        for j in range(MJ):
                            ps = ps1[j]
                            for half, mj in ((1, j + MJ), (0, j)):
                                dst = ps[:, half * Bs : (half + 1) * Bs]
                                msl = slice(mj * 128, (mj + 1) * 128)
                                mm(dst, tw1c1[:, msl], tx1[:], start=False,
                                   stop=(half == 0))
                            sg = sig_pool.tile([128, Bs], f32, tag=SIG1T)
                            nc.scalar.activation(
                                sg[:], ps[:, Bs : 2 * Bs], SIGMOID,
                                bias=tb1b[:, j : j + 1]
                            )
                            ht = h_pool.tile([128, Bs], f32r, tag=H1T)
                            nc.vector.scalar_tensor_tensor(
                                ht[:], ps[:, 0:Bs], tb1a[:, j : j + 1], sg[:],
                                ALU.add, ALU.mult,
                            )
                            h1.append(ht)
                        h2 = []
                        for j in range(MJ):
                            ps = ps_pool.tile([128, 2 * Bs], f32, tag="psmm")
                            # b-half group first so the sigmoid overlaps the
                            # a-half matmuls (keeps ACT off the eval tail)
                            dstb = ps[:, Bs : 2 * Bs]
                            for kc in range(KC):
                                csl = slice(kc * 2 * H + (j + MJ) * 128,
                                            kc * 2 * H + (j + MJ + 1) * 128)
                                mm(dstb, tw2[:, csl], h1[kc][:],
                                   start=(kc == 0), stop=(kc == KC - 1))
                            sg = sig_pool.tile([128, Bs], f32, tag=SIG2T)
                            nc.scalar.activation(
                                sg[:], dstb, SIGMOID, bias=tb2b[:, j : j + 1]
                            )
                            dsta = ps[:, 0:Bs]
                            for kc in range(KC):
                                csl = slice(kc * 2 * H + j * 128,
                                            kc * 2 * H + (j + 1) * 128)
                                mm(dsta, tw2[:, csl], h1[kc][:],
                                   start=(kc == 0), stop=(kc == KC - 1))
                            ht = h_pool.tile([128, Bs], f32r, tag=H2T)
                            nc.vector.scalar_tensor_tensor(
                                ht[:], dsta, tb2a[:, j : j + 1], sg[:],
                                ALU.add, ALU.mult,
                            )
                            h2.append(ht)
                        if not last_eval:
                            ps1_next = issue_l1ctx()
                    elif SCHED == 0:
                        # v1.5: per-j, both halves, GLU immediately
                        h1 = []
                        for j in range(MJ):
                            ps = ps_pool.tile([128, 2 * Bs], f32, tag="psmm")
                            for half, mj in ((0, j), (1, j + MJ)):
                                dst = ps[:, half * Bs : (half + 1) * Bs]
                                msl = slice(mj * 128, (mj + 1) * 128)
                                mm(dst, tw1c2[:, msl], tx2[:], start=True, stop=False)
                                mm(dst, tw1c1[:, msl], tx1[:], start=False, stop=True)
                            sg = sig_pool.tile([128, Bs], f32, tag="sig")
                            nc.scalar.activation(
                                sg[:], ps[:, Bs : 2 * Bs], SIGMOID,
                                bias=tb1b[:, j : j + 1]
                            )
                            ht = h_pool.tile([128, Bs], f32r, tag="hact")
                            nc.vector.scalar_tensor_tensor(
                                ht[:], ps[:, 0:Bs], tb1a[:, j : j + 1], sg[:],
                                ALU.add, ALU.mult,
                            )
                            h1.append(ht)
                        h2 = []
                        for j in range(MJ):
                            ps = ps_pool.tile([128, 2 * Bs], f32, tag="psmm")
                            for half, mj in ((0, j), (1, j + MJ)):
                                dst = ps[:, half * Bs : (half + 1) * Bs]
                                for kc in range(KC):
                                    csl = slice(kc * 2 * H + mj * 128,
                                                kc * 2 * H + (mj + 1) * 128)
                                    mm(dst, tw2[:, csl], h1[kc][:],
                                       start=(kc == 0), stop=(kc == KC - 1))
                            sg = sig_pool.tile([128, Bs], f32, tag="sig")
                            nc.scalar.activation(
                                sg[:], ps[:, Bs : 2 * Bs], SIGMOID,
                                bias=tb2b[:, j : j + 1]
                            )
                            ht = h_pool.tile([128, Bs], f32r, tag="hact")
                            nc.vector.scalar_tensor_tensor(
                                ht[:], ps[:, 0:Bs], tb2a[:, j : j + 1], sg[:],
                                ALU.add, ALU.mult,
                            )
                            h2.append(ht)
                    else:
                        if SCHED < 2:
                            ps1 = issue_l1ctx()
                        sg1 = []
                        for j in range(MJ):
                            dst = ps1[j][:, Bs : 2 * Bs]
                            msl = slice((j + MJ) * 128, (j + MJ + 1) * 128)
                            mm(dst, tw1c1[:, msl], tx1[:], start=False, stop=True)
                            sg = sig_pool.tile([128, Bs], f32, tag="sig")
                            nc.scalar.activation(
                                sg[:], dst, SIGMOID, bias=tb1b[:, j : j + 1]
                            )
                            sg1.append(sg)
                        h1 = []
                        for j in range(MJ):
                            dst = ps1[j][:, 0:Bs]
                            msl = slice(j * 128, (j + 1) * 128)
                            mm(dst, tw1c1[:, msl], tx1[:], start=False, stop=True)
                            ht = h_pool.tile([128, Bs], f32r, tag="hact")
                            nc.vector.scalar_tensor_tensor(
                                ht[:], dst, tb1a[:, j : j + 1], sg1[j][:],
                                ALU.add, ALU.mult,
                            )
                            h1.append(ht)
                        ps2 = []
                        for j in range(MJ):
                            ps2j = ps_pool.tile([128, 2 * Bs], f32, tag="psmm")
                            ps2.append(ps2j)
                        for kc in range(KC - 1):
                            for j in range(MJ):
                                csl = slice(kc * 2 * H + (j + MJ) * 128,
                                            kc * 2 * H + (j + MJ + 1) * 128)
                                mm(ps2[j][:, Bs : 2 * Bs], tw2[:, csl], h1[kc][:],
                                   start=(kc == 0), stop=False)
                            for j in range(MJ):
                                csl = slice(kc * 2 * H + j * 128,
                                            kc * 2 * H + (j + 1) * 128)
                                mm(ps2[j][:, 0:Bs], tw2[:, csl], h1[kc][:],
                                   start=(kc == 0), stop=False)
                        kc = KC - 1
                        sg2 = []
                        for j in range(MJ):
                            csl = slice(kc * 2 * H + (j + MJ) * 128,
                                        kc * 2 * H + (j + MJ + 1) * 128)
                            dst = ps2[j][:, Bs : 2 * Bs]
                            mm(dst, tw2[:, csl], h1[kc][:], start=False, stop=True)
                            sg = sig_pool.tile([128, Bs], f32, tag="sig")
                            nc.scalar.activation(
                                sg[:], dst, SIGMOID, bias=tb2b[:, j : j + 1]
                            )
                            sg2.append(sg)
                        h2 = []
                        for j in range(MJ):
                            csl = slice(kc * 2 * H + j * 128,
                                        kc * 2 * H + (j + 1) * 128)
                            dst = ps2[j][:, 0:Bs]
                            mm(dst, tw2[:, csl], h1[kc][:], start=False, stop=True)
                            ht = h_pool.tile([128, Bs], f32r, tag="hact")
                            nc.vector.scalar_tensor_tensor(
                                ht[:], dst, tb2a[:, j : j + 1], sg2[j][:],
                                ALU.add, ALU.mult,
                            )
                            h2.append(ht)
                        if SCHED >= 2 and not last_eval:
                            ps1_next = issue_l1ctx()

                    # ---- layer 3: k = h2 @ W3 (+ b3) in PSUM ----
                    if PS3_SHARE:
                        ps3full = ps_pool.tile([128, 2 * Bs], f32, tag="psmm")
                        ps3 = ps3full[0:D, 0:Bs]
                    else:
                        ps3 = ps3_pool.tile([D, Bs], f32, tag="ps3")
                    for kc in range(KC):
                        mm(ps3[:], tw3[:, kc * D : (kc + 1) * D], h2[kc][:],
                           start=(kc == 0), stop=(kc == KC - 1 and b3_is_zero))
                    if not b3_is_zero:
                        mm(ps3[:], tb3[:], tones[:], start=False, stop=True)

                    # ---- RK4 bookkeeping ----
                    if TMEMSET_POOL_TAIL and not last_eval:
                        nxt_s, nxt_e = (s, e + 1) if e < 3 else (s + 1, 0)
                        nxt_t = (nxt_s + TOFF[nxt_e]) * dt
                        if nxt_t != t_written:
                            nc.gpsimd.memset(
                                tx1[D : D + 1, :].bitcast(f32), float(nxt_t)
                            )
                            t_written = nxt_t
                    if TMEMSET_TAIL and not last_eval:
                        # write the NEXT eval's t-row on DVE (same engine as
                        # the arg STT -> no extra cross-engine hop on the
                        # arg -> layer-1 chain)
                        nxt_s, nxt_e = (s, e + 1) if e < 3 else (s + 1, 0)
                        nxt_t = (nxt_s + TOFF[nxt_e]) * dt
                        if nxt_t != t_written:
                            nc.vector.memset(
                                tx1[D : D + 1, :].bitcast(f32), float(nxt_t)
                            )
                            t_written = nxt_t
                    if e < 3:
                        nc.vector.scalar_tensor_tensor(
                            tx1[0:D, :], ps3[:], float(c_arg[e]), th_cur[:],
                            ALU.mult, ALU.add,
                        )
                    if TMEMSET_POST and not last_eval:
                        nxt_s, nxt_e = (s, e + 1) if e < 3 else (s + 1, 0)
                        nxt_t = (nxt_s + TOFF[nxt_e]) * dt
                        if nxt_t != t_written:
                            nc.vector.memset(
                                tx1[D : D + 1, :].bitcast(f32), float(nxt_t)
                            )
                            t_written = nxt_t
                    base = th_cur if e == 0 else acc_prev
                    if e == 3 and s != steps - 1:
                        # theta_{s+1} goes straight into the matmul input tile
                        # (keeps the Pool copy off the critical chain)...
                        nc.vector.scalar_tensor_tensor(
                            tx1[0:D, :], ps3[:], float(a_acc[e]), base[:],
                            ALU.mult, ALU.add,
                        )
                    acc_new = acc_pool.tile([D, Bs], f32, tag="accp")
                    # ...and also into its own tile (used as th_cur next step)
                    nc.vector.scalar_tensor_tensor(
                        acc_new[:], ps3[:], float(a_acc[e]), base[:],
                        ALU.mult, ALU.add,
                    )
                    acc_prev = acc_new
                    if SCHED >= 2 and not last_eval:
                        ps1 = ps1_next

                th_cur = acc_prev  # theta_{s+1}

            nc.sync.dma_start(d_out[:], th_cur[:])

    # ---- per-core input maps ----
    in_maps = []
    for c in range(N_CORES):
        sl = slice(c * Bs, (c + 1) * Bs)
        th_T = np.ascontiguousarray(np.asarray(theta0[sl], np.float32).T)
        ctx_T = np.ascontiguousarray(np.asarray(context[sl], np.float32).T)
        x1i = np.concatenate([th_T, np.zeros((1, Bs), np.float32)], axis=0)
        in_maps.append(
            {
                "x1i": np.ascontiguousarray(x1i),
                "x2i": ctx_T,
                "th0": th_T,
                "w1c1": w1c1_h,
                "w1c2": w1c2_h,
                "w2": w2_h,
                "w3": w3_h,
                "biases": bias_h,
                "onesb3": onesb3_h,
            }
        )

    return nc, in_maps


def _build_and_run(theta0, context, W1, b1, W2, b2, W3, b3, n_steps):
    from concourse.bass_utils import run_bass_kernel_spmd

    nc, in_maps = _build_program(theta0, context, W1, b1, W2, b2, W3, b3, n_steps)
    nc.finalize()  # Bacc: split multi-sem waits + allocate registers
    res = run_bass_kernel_spmd(
        nc,
        in_maps,
        core_ids=list(range(N_CORES)),
        trace=bool(int(os.environ.get("KERNEL_TRACE", "0"))),
    )
    _build_and_run.last_results = res

    out = np.concatenate([r["out"].T for r in res.results], axis=0)
    return np.ascontiguousarray(out.astype(np.float32))


def kernel(theta0, context, W1, b1, W2, b2, W3, b3, n_steps):
    return _build_and_run(
        np.asarray(theta0), np.asarray(context), W1, b1, W2, b2, W3, b3, n_steps
    )



# revision 37
# speedup vs baseline: 1.3517x; 1.0164x over previous
"""CCNF RK4 sampling kernel for 8 Trainium2 NeuronCores.

Data-parallel: batch 2048 -> 256 per core, weights replicated.
On-core layout: features on partitions, batch on the free dim (N=256).
Matmuls run in float32r (fp32 data, fast PE mode).
"""

import os

import numpy as np

N_CORES = 8


def _build_program_fast(theta0, context, W1, b1, W2, b2, W3, b3, n_steps):
    """Two-stream fast path (requires zero biases).

    Per core: batch 256 split into two independent 128-sample streams so
    each stream's sigmoid/GLU chain overlaps the other stream's matmuls.
    Layer 1 and 3 run in bf16, layer 2 in fp8e4m3 with DoubleRow (2x PE).
    """
    import ml_dtypes

    import concourse.mybir as mybir
    import concourse.tile as tile
    from concourse import bacc

    f32 = mybir.dt.float32
    bf16 = mybir.dt.bfloat16
    f8 = mybir.dt.float8e4
    ALU = mybir.AluOpType
    SIGMOID = mybir.ActivationFunctionType.Sigmoid
    DR = mybir.MatmulPerfMode.DoubleRow

    np_bf16 = ml_dtypes.bfloat16
    np_f8 = ml_dtypes.float8_e4m3

    B, D = theta0.shape          # 2048, 32
    C = context.shape[1]         # 128
    IN, H2 = W1.shape            # 161, 1024
    H = W2.shape[0]              # 512
    assert H2 == 2 * H and W2.shape[1] == 2 * H and W3.shape == (H, D)
    assert IN == D + 1 + C and D == 32 and C == 128 and H == 512
    assert B % (N_CORES * 2) == 0
    Bs = B // N_CORES            # 256 per core
    Ns = Bs // 2                 # 128 per stream
    steps = int(n_steps)
    dtv = 1.0 / steps

    S_H1 = 16.0                  # h1 tiles carry 16*h1 in fp8
    S_W2 = 32.0                  # W2 stored as 32*W2 in fp8

    # ---- host-side packing (shared across cores) ----
    W1 = np.asarray(W1, np.float32)
    w1a_h = np.ascontiguousarray(W1[: D + 1]).astype(np_bf16)      # [33,1024]
    w1c_h = np.ascontiguousarray(W1[D + 1 :]).astype(np_bf16)      # [128,1024]
    # W2 -> DoubleRow pairs: w2p[p, i, g*2H + m] = 32*W2[g*256 + i*128 + p, m]
    w2 = np.asarray(W2, np.float32) * S_W2
    w2p_h = np.ascontiguousarray(
        w2.reshape(2, 2, 128, 2 * H).transpose(2, 1, 0, 3).reshape(128, 4 * 2 * H)
    ).astype(np_f8)                                                # [128, 8192]
    # W3 -> [128, KC*D]: cols kc*D+d = W3[kc*128+p, d]
    w3_h = np.ascontiguousarray(
        np.asarray(W3, np.float32).reshape(4, 128, D).transpose(1, 0, 2).reshape(128, 4 * D)
    ).astype(np_bf16)

    c_arg = [0.5 * dtv, 0.5 * dtv, dtv]
    a_acc = [dtv / 6.0, dtv / 3.0, dtv / 3.0, dtv / 6.0]
    TOFF = (0.0, 0.5, 0.5, 1.0)

    nc = bacc.Bacc("TRN2", target_bir_lowering=False)

    d_x1 = nc.dram_tensor("x1", [D + 1, Bs], bf16, kind="ExternalInput")
    d_ctx = nc.dram_tensor("ctx", [C, Bs], bf16, kind="ExternalInput")
    d_th0 = nc.dram_tensor("th0", [D, Bs], f32, kind="ExternalInput")
    d_w1a = nc.dram_tensor("w1a", [D + 1, 2 * H], bf16, kind="ExternalInput")
    d_w1c = nc.dram_tensor("w1c", [C, 2 * H], bf16, kind="ExternalInput")
    d_w2p = nc.dram_tensor("w2p", [128, 4 * 2 * H], f8, kind="ExternalInput")
    d_w3 = nc.dram_tensor("w3", [128, 4 * D], bf16, kind="ExternalInput")
    d_out = nc.dram_tensor("out", [D, Bs], f32, kind="ExternalOutput")

    with tile.TileContext(nc) as tc:
        
        with (
            tc.tile_pool(name="const", bufs=1) as cpool,
            tc.tile_pool(name="psl1", bufs=3, space="PSUM") as ps1_pool,
            tc.tile_pool(name="psl2", bufs=3, space="PSUM") as ps2_pool,
            tc.tile_pool(name="ps3a", bufs=1, space="PSUM") as ps3a_pool,
            tc.tile_pool(name="ps3b", bufs=1, space="PSUM") as ps3b_pool,
            tc.tile_pool(name="sig", bufs=int(os.environ.get("KERNEL_SIGB", "8"))) as sig_pool,
            tc.tile_pool(name="hact", bufs=int(os.environ.get("KERNEL_HB", "12"))) as h_pool,
            tc.tile_pool(name="accp", bufs=int(os.environ.get("KERNEL_AB", "8"))) as acc_pool,
        ):
            tw1a = cpool.tile([D + 1, 2 * H], bf16)
            tw1c = cpool.tile([C, 2 * H], bf16)
            tw2p = cpool.tile([128, 2, 2 * 2 * H], f8)  # [p, i, g*2H+m]
            tw3 = cpool.tile([128, 4 * D], bf16)
            tctx = cpool.tile([C, Bs], bf16)            # both streams
            txp = [[cpool.tile([D + 1, Ns], bf16, name=f"txp{u}_{p}") for p in range(2)]
                   for u in range(2)]
            tth0 = cpool.tile([D, Bs], f32)
            twz = cpool.tile([1, 128], bf16)
            tout = cpool.tile([D, Bs], f32)

            # warmup: burn the PE p-state ramp while input DMAs stream
            nc.gpsimd.memset(twz[:], 0.0)
            NWARM = int(os.environ.get("KERNEL_WARM", "24"))
            if NWARM:
                psw = ps1_pool.tile([128, 2 * Bs], f32, tag="psmm", name="psw")
                for i in range(NWARM):
                    nc.tensor.matmul(psw[:, 0:128], twz[:], twz[:],
                                     start=(i == 0), stop=(i == NWARM - 1))

            # l1-critical tensors first (dispatch order == arrival order)
            nc.sync.dma_start(tctx[:], d_ctx[:])
            nc.sync.dma_start(tw1c[:], d_w1c[:])
            nc.sync.dma_start(txp[0][0][:], d_x1[:, 0:Ns])
            nc.sync.dma_start(txp[1][0][:], d_x1[:, Ns:Bs])
            nc.sync.dma_start(tw1a[:], d_w1a[:])
            nc.sync.dma_start(tw2p[:], d_w2p[:])
            nc.sync.dma_start(tw3[:], d_w3[:])
            nc.sync.dma_start(tth0[:], d_th0[:])

            def mm(out_ap, lhsT_ap, rhs_ap, start, stop, pm=None):
                nc.tensor.matmul(out_ap, lhsT_ap, rhs_ap, start=start,
                                 stop=stop, perf_mode=pm)

            # W1 column base per bank: A01, A23, B01, B23
            L1_BANKS = ((0, "A01"), (256, "A23"), (512, "B01"), (768, "B23"))

            th_cur = [tth0[:, 0:Ns], tth0[:, Ns:Bs]]
            acc_prev = [None, None]

            def alloc_l1_banks():
                bb, ba = [], []
                for u in range(2):
                    tb = ps1_pool.tile([128, 2 * Bs], f32, tag="psmm", name=f"l1B{u}")
                    ta = ps1_pool.tile([128, 2 * Bs], f32, tag="psmm", name=f"l1A{u}")
                    bb.append(tb)
                    ba.append(ta)
                return bb, ba

            def emit_ctx(bb, ba):
                # ctx half of layer 1 for the NEXT eval: tx-independent,
                # fills PE idle during the current eval's tail
                for u in range(2):
                    ctxu = tctx[:, u * Ns : (u + 1) * Ns]
                    for jb in range(4):
                        mm(bb[u][:, jb * 128 : (jb + 1) * 128],
                           tw1c[:, H + jb * 128 : H + (jb + 1) * 128],
                           ctxu, start=(jb == 0), stop=False)
                    for ja in range(4):
                        mm(ba[u][:, ja * 128 : (ja + 1) * 128],
                           tw1c[:, ja * 128 : (ja + 1) * 128],
                           ctxu, start=(ja == 0), stop=False)

            nbanks = alloc_l1_banks()
            emit_ctx(*nbanks)

            for s in range(steps):
                for e in range(4):
                    last_eval = (s == steps - 1) and (e == 3)
                    par = (4 * s + e) % 2
                    bankB, bankA = nbanks

                    # ---- layer 1 theta MMs + GLU per stream
                    hp = [None, None]
                    for u in range(2):
                        for jb in range(4):
                            mm(bankB[u][:, jb * 128 : (jb + 1) * 128],
                               tw1a[:, H + jb * 128 : H + (jb + 1) * 128],
                               txp[u][par][:], start=False, stop=(jb == 3))
                        sg1 = sig_pool.tile([128, 2 * Bs], f32, tag=f"sg{u}", name=f"sg1_{u}")
                        nc.scalar.activation(sg1[:, 0 : 2 * Bs],
                                             bankB[u][:, 0 : 2 * Bs], SIGMOID)
                        for ja in range(4):
                            mm(bankA[u][:, ja * 128 : (ja + 1) * 128],
                               tw1a[:, ja * 128 : (ja + 1) * 128],
                               txp[u][par][:], start=False, stop=(ja == 3))
                        ht = h_pool.tile([128, 4, 128], f8, tag=f"hp{u}", name=f"hp{u}")
                        nc.vector.scalar_tensor_tensor(
                            ht[:, :, :], bankA[u][:, 0 : 2 * Bs], S_H1,
                            sg1[:, 0 : 2 * Bs], ALU.mult, ALU.mult,
                        )
                        hp[u] = ht

                    # ---- layer 2 per stream
                    hq = [None, None]
                    bases = [th_cur[u] if e == 0 else acc_prev[u] for u in range(2)]
                    for u in range(2):
                        b2B = ps2_pool.tile([128, 2 * Bs], f32, tag="psmm", name=f"l2B{u}")
                        b2A = ps2_pool.tile([128, 2 * Bs], f32, tag="psmm", name=f"l2A{u}")
                        for g in range(2):
                            for jb in range(4):
                                mj = 4 + jb
                                csl = slice(g * 2 * H + mj * 128, g * 2 * H + (mj + 1) * 128)
                                mm(b2B[:, jb * 128 : (jb + 1) * 128],
                                   tw2p[:, :, csl], hp[u][:, 2 * g : 2 * g + 2, :],
                                   start=(g == 0 and jb == 0), stop=(g == 1 and jb == 3),
                                   pm=DR)
                        sg2 = sig_pool.tile([128, 2 * Bs], f32, tag=f"s2{u}", name=f"sg2_{u}")
                        nc.scalar.activation(
                            sg2[:, 0 : 2 * Bs], b2B[:, 0 : 2 * Bs], SIGMOID,
                            scale=1.0 / (S_H1 * S_W2),
                        )
                        for g in range(2):
                            for ja in range(4):
                                csl = slice(g * 2 * H + ja * 128, g * 2 * H + (ja + 1) * 128)
                                mm(b2A[:, ja * 128 : (ja + 1) * 128],
                                   tw2p[:, :, csl], hp[u][:, 2 * g : 2 * g + 2, :],
                                   start=(g == 0 and ja == 0), stop=(g == 1 and ja == 3),
                                   pm=DR)
                        ht = h_pool.tile([128, 4, 128], bf16, tag=f"hq{u}", name=f"hq{u}")
                        nc.vector.scalar_tensor_tensor(
                            ht[:, :, :], b2A[:, 0 : 2 * Bs],
                            1.0 / (S_H1 * S_W2), sg2[:, 0 : 2 * Bs],
                            ALU.mult, ALU.mult,
                        )
                        hq[u] = ht

                    # ---- ctx MMs of the NEXT eval (PE filler during tail)
                    if not last_eval:
                        nbanks = alloc_l1_banks()
                        emit_ctx(*nbanks)

                    # ---- layer 3 + tx per stream
                    ps3s = [None, None]
                    for u in range(2):
                        ps3s[u] = (ps3a_pool if u == 0 else ps3b_pool).tile(
                            [D, Ns], f32, tag=f"ps3{u}", name=f"ps3{u}")
                        for kc in range(4):
                            mm(ps3s[u][:],
                               tw3[:, kc * D : (kc + 1) * D],
                               hq[u][:, kc : kc + 1, :],
                               start=(kc == 0), stop=(kc == 3))
                        nxt = txp[u][1 - par]
                        if e < 3:
                            nc.vector.scalar_tensor_tensor(
                                nxt[0:D, :], ps3s[u][:], float(c_arg[e]), th_cur[u],
                                ALU.mult, ALU.add,
                            )
                        elif not last_eval:
                            nc.vector.scalar_tensor_tensor(
                                nxt[0:D, :], ps3s[u][:], float(a_acc[e]), bases[u],
                                ALU.mult, ALU.add,
                            )
                        if not last_eval:
                            nxt_s, nxt_e = (s, e + 1) if e < 3 else (s + 1, 0)
                            nxt_t = (nxt_s + TOFF[nxt_e]) * dtv
                            nc.gpsimd.memset(nxt[D : D + 1, :], float(nxt_t))

                    for u in range(2):
                        if last_eval:
                            acc_new = tout[:, u * Ns : (u + 1) * Ns]
                        else:
                            acc_new = acc_pool.tile(
                                [D, Ns], f32, tag=f"acc{u}", name=f"acc{u}")[:]
                        nc.vector.scalar_tensor_tensor(
                            acc_new, ps3s[u][:], float(a_acc[e]), bases[u],
                            ALU.mult, ALU.add,
                        )
                        acc_prev[u] = acc_new

                for u in range(2):
                    th_cur[u] = acc_prev[u]

            nc.sync.dma_start(d_out[:], tout[:])

    # ---- per-core input maps ----
    in_maps = []
    for c in range(N_CORES):
        sl = slice(c * Bs, (c + 1) * Bs)
        th_T = np.ascontiguousarray(np.asarray(theta0[sl], np.float32).T)
        ctx_T = np.ascontiguousarray(np.asarray(context[sl], np.float32).T)
        x1 = np.concatenate([th_T, np.zeros((1, Bs), np.float32)], axis=0)
        in_maps.append(
            {
                "x1": np.ascontiguousarray(x1.astype(np_bf16)),
                "ctx": np.ascontiguousarray(ctx_T.astype(np_bf16)),
                "th0": th_T,
                "w1a": w1a_h,
                "w1c": w1c_h,
                "w2p": w2p_h,
                "w3": w3_h,
            }
        )
    return nc, in_maps


def _build_program(theta0, context, W1, b1, W2, b2, W3, b3, n_steps):
    if (
        int(os.environ.get("KERNEL_FAST", "1"))
        and not np.any(np.asarray(b1))
        and not np.any(np.asarray(b2))
        and not np.any(np.asarray(b3))
        and theta0.shape == (2048, 32)
        and int(n_steps) == 16
    ):
        return _build_program_fast(
            theta0, context, W1, b1, W2, b2, W3, b3, n_steps
        )
    return _build_program_v1(theta0, context, W1, b1, W2, b2, W3, b3, n_steps)


def _build_program_v1(theta0, context, W1, b1, W2, b2, W3, b3, n_steps):
    import concourse.bass as bass
    import concourse.mybir as mybir
    import concourse.tile as tile
    from concourse import bacc
    from concourse.bass_utils import run_bass_kernel_spmd

    f32 = mybir.dt.float32
    f32r = mybir.dt.float32r
    ALU = mybir.AluOpType
    SIGMOID = mybir.ActivationFunctionType.Sigmoid

    B, D = theta0.shape          # 2048, 32
    C = context.shape[1]         # 128
    IN, H2 = W1.shape            # 161, 1024
    H = W2.shape[0]              # 512
    assert H2 == 2 * H and W2.shape[1] == 2 * H and W3.shape == (H, D)
    assert IN == D + 1 + C
    assert B % N_CORES == 0
    Bs = B // N_CORES            # 256 per core
    steps = int(n_steps)
    dt = 1.0 / steps

    KC = H // 128                # 4 k-chunks for layer 2/3
    MJ = H // 128                # 4 output column-chunks per GLU half
    # layer-1 K split: rows [0:33) = theta(32)+t(1) (dynamic); rows [33:161) = ctx (static)
    K1A = D + 1                  # 33 (theta + t)
    K1B = IN - K1A               # 128 (ctx)

    # ---- host-side layout prep (shared across cores) ----
    W1 = np.ascontiguousarray(W1, np.float32)
    w1c1_h = np.ascontiguousarray(W1[:K1A])                    # [33, 1024]  theta+t rows
    w1c2_h = np.ascontiguousarray(W1[K1A:])                    # [128, 1024] ctx rows
    b3_is_zero = not np.any(np.asarray(b3, np.float32))
    # [512,1024] -> [128, 4*1024]: chunk kc at cols kc*1024
    w2_h = np.ascontiguousarray(
        np.asarray(W2, np.float32).reshape(KC, 128, 2 * H).transpose(1, 0, 2).reshape(128, KC * 2 * H)
    )
    # [512,32] -> [128, 4*32]
    w3_h = np.ascontiguousarray(
        np.asarray(W3, np.float32).reshape(KC, 128, D).transpose(1, 0, 2).reshape(128, KC * D)
    )
    b1 = np.asarray(b1, np.float32)
    b2 = np.asarray(b2, np.float32)
    bias_h = np.ascontiguousarray(np.concatenate([
        b1[:H].reshape(MJ, 128).T, b1[H:].reshape(MJ, 128).T,
        b2[:H].reshape(MJ, 128).T, b2[H:].reshape(MJ, 128).T,
    ], axis=1))                                                # [128, 16]
    onesb3_h = np.ascontiguousarray(np.concatenate([
        np.ones((1, Bs), np.float32),
        np.asarray(b3, np.float32).reshape(1, D),
    ], axis=1))                                                # [1, Bs+32]

    # ---- build the bass program (same program on all 8 cores) ----
    nc = bacc.Bacc("TRN2", target_bir_lowering=False)

    d_x1i = nc.dram_tensor("x1i", [K1A, Bs], f32r, kind="ExternalInput")  # theta rows + t row
    d_x2i = nc.dram_tensor("x2i", [K1B, Bs], f32r, kind="ExternalInput")  # ctx (static)
    d_th0 = nc.dram_tensor("th0", [D, Bs], f32r, kind="ExternalInput")
    d_w1c1 = nc.dram_tensor("w1c1", [K1A, 2 * H], f32r, kind="ExternalInput")
    d_w1c2 = nc.dram_tensor("w1c2", [K1B, 2 * H], f32r, kind="ExternalInput")
    d_w2 = nc.dram_tensor("w2", [128, KC * 2 * H], f32r, kind="ExternalInput")
    d_w3 = nc.dram_tensor("w3", [128, KC * D], f32r, kind="ExternalInput")
    d_bias = nc.dram_tensor("biases", [128, 4 * MJ], f32, kind="ExternalInput")
    d_ob3 = nc.dram_tensor("onesb3", [1, Bs + D], f32r, kind="ExternalInput")
    d_out = nc.dram_tensor("out", [D, Bs], f32, kind="ExternalOutput")

    # RK4 coefficients: arg scale (for next eval's input), acc scale
    c_arg = [0.5 * dt, 0.5 * dt, dt]
    a_acc = [dt / 6.0, dt / 3.0, dt / 3.0, dt / 6.0]

    with tile.TileContext(nc) as tc:
        PS3_SHARE = int(os.environ.get("KERNEL_PS3SHARE", "0"))
        PSMM_BUFS = 8 if PS3_SHARE else 7
        with (
            tc.tile_pool(name="const", bufs=1) as cpool,
            tc.tile_pool(name="psmm", bufs=PSMM_BUFS, space="PSUM") as ps_pool,
            tc.tile_pool(name="ps3a", bufs=1, space="PSUM") as ps3a_pool,
            tc.tile_pool(name="ps3b", bufs=1, space="PSUM") as ps3b_pool,
            tc.tile_pool(name="sig", bufs=int(os.environ.get("KERNEL_SIGB", "10"))) as sig_pool,
            tc.tile_pool(name="hact", bufs=int(os.environ.get("KERNEL_HB", "20"))) as h_pool,
            tc.tile_pool(name="accp", bufs=int(os.environ.get("KERNEL_AB", "6"))) as acc_pool,
        ):
            tw1c1 = cpool.tile([K1A, 2 * H], f32r)
            tw1c2 = cpool.tile([K1B, 2 * H], f32r)
            tw2 = cpool.tile([128, KC * 2 * H], f32r)
            tw3 = cpool.tile([128, KC * D], f32r)
            tbias = cpool.tile([128, 4 * MJ], f32)
            tb1a = tbias[:, 0 * MJ : 1 * MJ]
            tb1b = tbias[:, 1 * MJ : 2 * MJ]
            tb2a = tbias[:, 2 * MJ : 3 * MJ]
            tb2b = tbias[:, 3 * MJ : 4 * MJ]
            tob3 = cpool.tile([1, Bs + D], f32r)
            tones = tob3[:, 0:Bs]
            tb3 = tob3[:, Bs : Bs + D]
            tx1 = cpool.tile([K1A, Bs], f32r)   # rows: [theta(32) | t(1)]  (dynamic)
            tx2 = cpool.tile([K1B, Bs], f32r)   # ctx (static)
            tth0 = cpool.tile([D, Bs], f32r)    # initial theta

            # layer-1-critical tensors first so eval 0 can start while
            # w2/w3 still stream
            nc.sync.dma_start(tx2[:], d_x2i[:])
            nc.sync.dma_start(tw1c2[:], d_w1c2[:])
            nc.sync.dma_start(tx1[:], d_x1i[:])
            nc.sync.dma_start(tw1c1[:], d_w1c1[:])
            nc.sync.dma_start(tbias[:], d_bias[:])
            nc.sync.dma_start(tth0[:], d_th0[:])
            nc.sync.dma_start(tw2[:], d_w2[:])
            nc.sync.dma_start(tw3[:], d_w3[:])
            nc.sync.dma_start(tob3[:], d_ob3[:])

            def mm(out_ap, lhsT_ap, rhs_ap, start, stop):
                nc.tensor.matmul(out_ap, lhsT_ap, rhs_ap, start=start, stop=stop)

            th_cur = tth0       # theta at start of current step
            t_written = 0.0     # t-row was preloaded with 0

            def issue_l1ctx():
                # static context contribution for the NEXT eval's layer 1 --
                # issued early so PE has work during the RK4 latency chain.
                # One accumulation group per PSUM bank: only the first MM may
                # carry start=True (a second start would zero the whole bank).
                tiles = []
                for j in range(MJ):
                    ps = ps_pool.tile([128, 2 * Bs], f32, tag="psmm")
                    for half, mj in ((1, j + MJ), (0, j)):
                        dst = ps[:, half * Bs : (half + 1) * Bs]
                        msl = slice(mj * 128, (mj + 1) * 128)
                        mm(dst, tw1c2[:, msl], tx2[:],
                           start=(half == 1), stop=False)
                    tiles.append(ps)
                return tiles

            SCHED = int(os.environ.get("KERNEL_SCHED", "3"))
            SPLITP = int(os.environ.get("KERNEL_SPLITPOOLS", "1"))
            SIG1T, SIG2T = ("sig1", "sig2") if SPLITP else ("sig", "sig")
            H1T, H2T = ("h1t", "h2t") if SPLITP else ("hact", "hact")
            if SCHED >= 2 or SCHED == 3:
                ps1 = issue_l1ctx()

            TOFF = (0.0, 0.5, 0.5, 1.0)
            for s in range(steps):
                for e in range(4):
                    TMEMSET_MODE = int(os.environ.get("KERNEL_TMEMSET", "0")) if SCHED == 3 else 0
                    TMEMSET_TAIL = TMEMSET_MODE == 1
                    TMEMSET_POST = TMEMSET_MODE == 2
                    TMEMSET_POOL_TAIL = TMEMSET_MODE == 3
                    t_val = (s + TOFF[e]) * dt
                    if TMEMSET_MODE == 0 and t_val != t_written:
                        nc.gpsimd.memset(tx1[D : D + 1, :].bitcast(f32), float(t_val))
                        t_written = t_val

                    last_eval = (s == steps - 1) and (e == 3)

                    if SCHED == 3:
                        # v1.5 structure, ctx MMs pre-issued (ps1 tiles);
                        # group stop goes on the LAST MM of the bank (a-half)
                        h1 = []
                        for j in range(MJ):
                            ps = ps1[j]
                            for half, mj in ((1, j + MJ), (0, j)):
                                dst = ps[:, half * Bs : (half + 1) * Bs]
                                msl = slice(mj * 128, (mj + 1) * 128)
                                mm(dst, tw1c1[:, msl], tx1[:], start=False,
                                   stop=(half == 0))
                            sg = sig_pool.tile([128, Bs], f32, tag=SIG1T)
                            nc.scalar.activation(
                                sg[:], ps[:, Bs : 2 * Bs], SIGMOID,
                                bias=tb1b[:, j : j + 1]
                            )
                            ht = h_pool.tile([128, Bs], f32r, tag=H1T)
                            nc.vector.scalar_tensor_tensor(
                                ht[:], ps[:, 0:Bs], tb1a[:, j : j + 1], sg[:],
                                ALU.add, ALU.mult,
                            )
                            h1.append(ht)
                        h2 = []
                        for j in range(MJ):
                            ps = ps_pool.tile([128, 2 * Bs], f32, tag="psmm")
                            # b-half group first so the sigmoid overlaps the
                            # a-half matmuls (keeps ACT off the eval tail)
                            dstb = ps[:, Bs : 2 * Bs]
                            for kc in range(KC):
                                csl = slice(kc * 2 * H + (j + MJ) * 128,
                                            kc * 2 * H + (j + MJ + 1) * 128)
                                mm(dstb, tw2[:, csl], h1[kc][:],
                                   start=(kc == 0), stop=(kc == KC - 1))
                            sg = sig_pool.tile([128, Bs], f32, tag=SIG2T)
                            nc.scalar.activation(
                                sg[:], dstb, SIGMOID, bias=tb2b[:, j : j + 1]
                            )
                            dsta = ps[:, 0:Bs]
                            for kc in range(KC):
                                csl = slice(kc * 2 * H + j * 128,
                                            kc * 2 * H + (j + 1) * 128)
                                mm(dsta, tw2[:, csl], h1[kc][:],
                                   start=(kc == 0), stop=(kc == KC - 1))
                            ht = h_pool.tile([128, Bs], f32r, tag=H2T)
                            nc.vector.scalar_tensor_tensor(
                                ht[:], dsta, tb2a[:, j : j + 1], sg[:],
                                ALU.add, ALU.mult,
                            )
                            h2.append(ht)
                        if not last_eval:
                            ps1_next = issue_l1ctx()
                    elif SCHED == 0:
                        # v1.5: per-j, both halves, GLU immediately
                        h1 = []
                        for j in range(MJ):
                            ps = ps_pool.tile([128, 2 * Bs], f32, tag="psmm")
                            for half, mj in ((0, j), (1, j + MJ)):
                                dst = ps[:, half * Bs : (half + 1) * Bs]
                                msl = slice(mj * 128, (mj + 1) * 128)
                                mm(dst, tw1c2[:, msl], tx2[:], start=True, stop=False)
                                mm(dst, tw1c1[:, msl], tx1[:], start=False, stop=True)
                            sg = sig_pool.tile([128, Bs], f32, tag="sig")
                            nc.scalar.activation(
                                sg[:], ps[:, Bs : 2 * Bs], SIGMOID,
                                bias=tb1b[:, j : j + 1]
                            )
                            ht = h_pool.tile([128, Bs], f32r, tag="hact")
                            nc.vector.scalar_tensor_tensor(
                                ht[:], ps[:, 0:Bs], tb1a[:, j : j + 1], sg[:],
                                ALU.add, ALU.mult,
                            )
                            h1.append(ht)
                        h2 = []
                        for j in range(MJ):
                            ps = ps_pool.tile([128, 2 * Bs], f32, tag="psmm")
                            for half, mj in ((0, j), (1, j + MJ)):
                                dst = ps[:, half * Bs : (half + 1) * Bs]
                                for kc in range(KC):
                                    csl = slice(kc * 2 * H + mj * 128,
                                                kc * 2 * H + (mj + 1) * 128)
                                    mm(dst, tw2[:, csl], h1[kc][:],
                                       start=(kc == 0), stop=(kc == KC - 1))
                            sg = sig_pool.tile([128, Bs], f32, tag="sig")
                            nc.scalar.activation(
                                sg[:], ps[:, Bs : 2 * Bs], SIGMOID,
                                bias=tb2b[:, j : j + 1]
                            )
                            ht = h_pool.tile([128, Bs], f32r, tag="hact")
                            nc.vector.scalar_tensor_tensor(
                                ht[:], ps[:, 0:Bs], tb2a[:, j : j + 1], sg[:],
                                ALU.add, ALU.mult,
                            )
                            h2.append(ht)
                    else:
                        if SCHED < 2:
                            ps1 = issue_l1ctx()
                        sg1 = []
                        for j in range(MJ):
                            dst = ps1[j][:, Bs : 2 * Bs]
                            msl = slice((j + MJ) * 128, (j + MJ + 1) * 128)
                            mm(dst, tw1c1[:, msl], tx1[:], start=False, stop=True)
                            sg = sig_pool.tile([128, Bs], f32, tag="sig")
                            nc.scalar.activation(
                                sg[:], dst, SIGMOID, bias=tb1b[:, j : j + 1]
                            )
                            sg1.append(sg)
                        h1 = []
                        for j in range(MJ):
                            dst = ps1[j][:, 0:Bs]
                            msl = slice(j * 128, (j + 1) * 128)
                            mm(dst, tw1c1[:, msl], tx1[:], start=False, stop=True)
                            ht = h_pool.tile([128, Bs], f32r, tag="hact")
                            nc.vector.scalar_tensor_tensor(
                                ht[:], dst, tb1a[:, j : j + 1], sg1[j][:],
                                ALU.add, ALU.mult,
                            )
                            h1.append(ht)
                        ps2 = []
                        for j in range(MJ):
                            ps2j = ps_pool.tile([128, 2 * Bs], f32, tag="psmm")
                            ps2.append(ps2j)
                        for kc in range(KC - 1):
                            for j in range(MJ):
                                csl = slice(kc * 2 * H + (j + MJ) * 128,
                                            kc * 2 * H + (j + MJ + 1) * 128)
                                mm(ps2[j][:, Bs : 2 * Bs], tw2[:, csl], h1[kc][:],
                                   start=(kc == 0), stop=False)
                            for j in range(MJ):
                                csl = slice(kc * 2 * H + j * 128,
                                            kc * 2 * H + (j + 1) * 128)
                                mm(ps2[j][:, 0:Bs], tw2[:, csl], h1[kc][:],
                                   start=(kc == 0), stop=False)
                        kc = KC - 1
                        sg2 = []
                        for j in range(MJ):
                            csl = slice(kc * 2 * H + (j + MJ) * 128,
                                        kc * 2 * H + (j + MJ + 1) * 128)
                            dst = ps2[j][:, Bs : 2 * Bs]
                            mm(dst, tw2[:, csl], h1[kc][:], start=False, stop=True)
                            sg = sig_pool.tile([128, Bs], f32, tag="sig")
                            nc.scalar.activation(
                                sg[:], dst, SIGMOID, bias=tb2b[:, j : j + 1]
                            )
                            sg2.append(sg)
                        h2 = []
                        for j in range(MJ):
                            csl = slice(kc * 2 * H + j * 128,
                                        kc * 2 * H + (j + 1) * 128)
                            dst = ps2[j][:, 0:Bs]
                            mm(dst, tw2[:, csl], h1[kc][:], start=False, stop=True)
                            ht = h_pool.tile([128, Bs], f32r, tag="hact")
                            nc.vector.scalar_tensor_tensor(
                                ht[:], dst, tb2a[:, j : j + 1], sg2[j][:],
                                ALU.add, ALU.mult,
                            )
                            h2.append(ht)
                        if SCHED >= 2 and not last_eval:
                            ps1_next = issue_l1ctx()

                    # ---- layer 3: k = h2 @ W3 (+ b3) in PSUM ----
                    if PS3_SHARE:
                        ps3full = ps_pool.tile([128, 2 * Bs], f32, tag="psmm")
                        ps3 = ps3full[0:D, 0:Bs]
                    else:
                        ps3 = ps3_pool.tile([D, Bs], f32, tag="ps3")
                    for kc in range(KC):
                        mm(ps3[:], tw3[:, kc * D : (kc + 1) * D], h2[kc][:],
                           start=(kc == 0), stop=(kc == KC - 1 and b3_is_zero))
                    if not b3_is_zero:
                        mm(ps3[:], tb3[:], tones[:], start=False, stop=True)

                    # ---- RK4 bookkeeping ----
                    if TMEMSET_POOL_TAIL and not last_eval:
                        nxt_s, nxt_e = (s, e + 1) if e < 3 else (s + 1, 0)
                        nxt_t = (nxt_s + TOFF[nxt_e]) * dt
                        if nxt_t != t_written:
                            nc.gpsimd.memset(
                                tx1[D : D + 1, :].bitcast(f32), float(nxt_t)
                            )
                            t_written = nxt_t
                    if TMEMSET_TAIL and not last_eval:
                        # write the NEXT eval's t-row on DVE (same engine as
                        # the arg STT -> no extra cross-engine hop on the
                        # arg -> layer-1 chain)
                        nxt_s, nxt_e = (s, e + 1) if e < 3 else (s + 1, 0)
                        nxt_t = (nxt_s + TOFF[nxt_e]) * dt
                        if nxt_t != t_written:
                            nc.vector.memset(
                                tx1[D : D + 1, :].bitcast(f32), float(nxt_t)
                            )
                            t_written = nxt_t
                    if e < 3:
                        nc.vector.scalar_tensor_tensor(
                            tx1[0:D, :], ps3[:], float(c_arg[e]), th_cur[:],
                            ALU.mult, ALU.add,
                        )
                    if TMEMSET_POST and not last_eval:
                        nxt_s, nxt_e = (s, e + 1) if e < 3 else (s + 1, 0)
                        nxt_t = (nxt_s + TOFF[nxt_e]) * dt
                        if nxt_t != t_written:
                            nc.vector.memset(
                                tx1[D : D + 1, :].bitcast(f32), float(nxt_t)
                            )
                            t_written = nxt_t
                    base = th_cur if e == 0 else acc_prev
                    if e == 3 and s != steps - 1:
                        # theta_{s+1} goes straight into the matmul input tile
                        # (keeps the Pool copy off the critical chain)...
                        nc.vector.scalar_tensor_tensor(
                            tx1[0:D, :], ps3[:], float(a_acc[e]), base[:],
                            ALU.mult, ALU.add,
                        )
                    acc_new = acc_pool.tile([D, Bs], f32, tag="accp")
                    # ...and also into its own tile (used as th_cur next step)
                    nc.vector.scalar_tensor_tensor(
                        acc_new[:], ps3[:], float(a_acc[e]), base[:],
                        ALU.mult, ALU.add,
                    )
                    acc_prev = acc_new
                    if SCHED >= 2 and not last_eval:
                        ps1 = ps1_next

                th_cur = acc_prev  # theta_{s+1}

            nc.sync.dma_start(d_out[:], th_cur[:])

    # ---- per-core input maps ----
    in_maps = []
    for c in range(N_CORES):
        sl = slice(c * Bs, (c + 1) * Bs)
        th_T = np.ascontiguousarray(np.asarray(theta0[sl], np.float32).T)
        ctx_T = np.ascontiguousarray(np.asarray(context[sl], np.float32).T)
        x1i = np.concatenate([th_T, np.zeros((1, Bs), np.float32)], axis=0)
        in_maps.append(
            {
                "x1i": np.ascontiguousarray(x1i),
                "x2i": ctx_T,
                "th0": th_T,
                "w1c1": w1c1_h,
                "w1c2": w1c2_h,
                "w2": w2_h,
                "w3": w3_h,
                "biases": bias_h,
                "onesb3": onesb3_h,
            }
        )

    return nc, in_maps


def _build_and_run(theta0, context, W1, b1, W2, b2, W3, b3, n_steps):
    from concourse.bass_utils import run_bass_kernel_spmd

    nc, in_maps = _build_program(theta0, context, W1, b1, W2, b2, W3, b3, n_steps)
    nc.finalize()  # Bacc: split multi-sem waits + allocate registers
    res = run_bass_kernel_spmd(
        nc,
        in_maps,
        core_ids=list(range(N_CORES)),
        trace=bool(int(os.environ.get("KERNEL_TRACE", "0"))),
    )
    _build_and_run.last_results = res

    out = np.concatenate([r["out"].T for r in res.results], axis=0)
    return np.ascontiguousarray(out.astype(np.float32))


def kernel(theta0, context, W1, b1, W2, b2, W3, b3, n_steps):
    return _build_and_run(
        np.asarray(theta0), np.asarray(context), W1, b1, W2, b2, W3, b3, n_steps
    )

